# revision 2
# baseline (speedup 1.0000x reference)
"""Trainium2 Bass kernel for 2D single-level DWT (coif1, symmetric padding).

Input  x: (4, 64, 512, 512) fp32
Output  : (4, 256, 258, 258) fp32  -- per input channel: [cA, cH, cV, cD]

Math: with R_f the banded 258x512 operator of the 1D DWT along an axis
(6-tap filter, stride 2, symmetric boundary folds), the four outputs are
    cA = R_lo X R_lo^T,  cH = R_hi X R_lo^T,
    cV = R_lo X R_hi^T,  cD = R_hi X R_hi^T.

On-device (per image, per core; 32 images per core, pure data-parallel):
  pass 1 (contract over rows r on the PE):   Yt_f[c, kh] = sum_r X[r, c] R_f[kh, r]
     matmul with lhsT = X column-slice (stationary), rhs = R_f^T chunk.
  pass 2 (contract over cols c on the PE):   O_s[kw, kh] = sum_c R_g[kw, c] Yt_f[c, kh]
     matmul with lhsT = R_g^T kw-slice (stationary), rhs = Yt_f chunk.
  Outputs land transposed ([kw, kh]); the host swaps the last two axes.

Matmuls run as float32r (1 cycle/row for N>=256, numerically fp32-width).
"""

import os
import sys

for _p in ("/opt/trn_rl_repo", "/opt/pypackages"):
    if _p not in sys.path:
        sys.path.append(_p)

os.environ.setdefault("JAX_COMPILATION_CACHE_DIR", "/tmp/jax_comp_cache")
os.environ.setdefault("JAX_PERSISTENT_CACHE_MIN_COMPILE_TIME_SECS", "10")

import numpy as np

import concourse.bass as bass
import concourse.bacc as bacc
import concourse.mybir as mybir
from concourse.bass_utils import run_bass_kernel_spmd
from concourse.tile import TileContext

N_CORES = 8
H = W = 512
OUT = 258  # (512 + 6 - 1) // 2
IMGS = 32  # images per core (4*64/8)
F32 = mybir.dt.float32
F32R = mybir.dt.float32r

# pywt coif1 decomposition filters (already flipped: correlation form)
DEC_LO = np.array([-0.01565572813546454, -0.0727326195128539, 0.38486484686420286,
                   0.8525720202122554, 0.3378976624578092, -0.0727326195128539])
DEC_HI = np.array([0.0727326195128539, 0.3378976624578092, -0.8525720202122554,
                   0.38486484686420286, 0.0727326195128539, -0.01565572813546454])
FLEN = 6
PAD = 4
LO_F = DEC_LO[::-1]
HI_F = DEC_HI[::-1]


def _build_R(filt: np.ndarray, n: int = W) -> np.ndarray:
    """Banded [258, 512] operator: out[k] = sum_j filt[j] * x[sym(2k + j - PAD)]."""
    out_len = (n + FLEN - 1) // 2

    def sym(i: int) -> int:
        while i < 0 or i >= n:
            if i < 0:
                i = -i - 1
            if i >= n:
                i = 2 * n - 1 - i
        return i

    R = np.zeros((out_len, n), dtype=np.float64)
    for k in range(out_len):
        for j in range(FLEN):
            R[k, sym(2 * k + j - PAD)] += filt[j]
    return R


def _build_weights() -> np.ndarray:
    """w[p, (f*4+q)*258 + k] = R_f[k, 128q + p], as [128, 8*258] fp32."""
    Rs = [_build_R(LO_F), _build_R(HI_F)]
    tiles = []
    for f in range(2):
        for q in range(4):
            tiles.append(Rs[f][:, 128 * q:128 * (q + 1)].T)
    stacked = np.stack(tiles)  # [8, 128, 258]
    w = np.ascontiguousarray(stacked.transpose(1, 0, 2).reshape(128, 8 * OUT)).astype(np.float32)
    return _round_tf32(w)


def _round_tf32(a: np.ndarray) -> np.ndarray:
    """Round-to-nearest-even to tf32 (10-bit mantissa), keeping fp32 storage."""
    bits = a.astype(np.float32).view(np.uint32)
    bits = (bits + 0xFFF + ((bits >> 13) & 1)) & np.uint32(0xFFFFE000)
    return bits.view(np.float32)


_WEIGHTS = _build_weights()
_MODULE = None
PS1_BUFS = 3
PS2_BUFS = 5
W_RING_SCALAR = False
XPOOL_BUFS = 2
YPOOL_BUFS = 2
SPOOL_BUFS = 2
OUT_SPLIT = 4
IN_SPLIT = 1


def _build_module() -> bass.Bass:
    nc = bacc.Bacc("TRN2", target_bir_lowering=False, debug=False)
    x_in = nc.declare_dram_parameter("x", [IMGS, H, W], F32R, isOutput=False)
    w_in = nc.declare_dram_parameter("w", [128, 8 * OUT], F32R, isOutput=False)
    # device layout: y[i, s, kw, kh] = O_s[kw, kh] (host swaps kh/kw)
    y_out = nc.declare_dram_parameter("y", [IMGS, 4, OUT, OUT], F32, isOutput=True)

    with TileContext(nc) as tc:
        with (
            tc.tile_pool(name="wpool", bufs=1) as wpool,
            tc.tile_pool(name="xpool", bufs=XPOOL_BUFS) as xpool,
            tc.tile_pool(name="ypool", bufs=YPOOL_BUFS) as ypool,
            tc.tile_pool(name="spool", bufs=SPOOL_BUFS) as spool,
            tc.tile_pool(name="psum", bufs=4, space="PSUM") as pspool,
        ):
            Wt = wpool.tile([128, 8 * OUT], F32R)
            # scalar-ring HWDGE so the weight load overlaps the first X load
            (nc.scalar if W_RING_SCALAR else nc.sync).dma_start(out=Wt[:], in_=w_in[:])
            Wr = Wt[:]

            # Tiny PE op consuming the weight DMA so later matmuls depend on
            # it via PE program order (Matmult carries at most one sync wait).
            warm = pspool.tile([1, OUT], F32, tag="ps2", bufs=PS2_BUFS)
            nc.tensor.matmul(warm[:, :], lhsT=Wr[:, 0:1], rhs=Wr[:, 0:OUT],
                             start=True, stop=True)

            def load_x(i):
                # X[p, q*512 + c] = x[i, 128q + p, c]
                X = xpool.tile([128, 4 * W], F32R, tag="X", name=f"X_{i}")
                xi = x_in[i].rearrange("(q p) c -> p q c", p=128)
                Xv = X.rearrange("p (q c) -> p q c", q=4)
                qper = 4 // IN_SPLIT
                for j in range(IN_SPLIT):
                    nc.sync.dma_start(
                        out=Xv[:, j * qper:(j + 1) * qper],
                        in_=xi[:, j * qper:(j + 1) * qper],
                    )
                return X

            ev = 0
            Xnext = load_x(0)
            for i in range(IMGS):
                Xr = Xnext[:]

                # pass 1: Yt[p, (f*4+cc)*258 + kh] = Yt_f[c = 128cc + p, kh]
                Yt = ypool.tile([128, 8 * OUT], F32R, tag="Yt")
                for f in range(2):
                    for cc in range(4):
                        ps = pspool.tile([128, OUT], F32, tag="ps1", bufs=PS1_BUFS)
                        for q in range(4):
                            nc.tensor.matmul(
                                ps[:, :],
                                lhsT=Xr[:, q * W + cc * 128: q * W + (cc + 1) * 128],
                                rhs=Wr[:, (f * 4 + q) * OUT: (f * 4 + q + 1) * OUT],
                                start=(q == 0),
                                stop=(q == 3),
                            )
                        dst = Yt[:, (f * 4 + cc) * OUT: (f * 4 + cc + 1) * OUT]
                        if ev % 2 == 0:
                            nc.scalar.copy(out=dst, in_=ps[:, :])
                        else:
                            nc.vector.tensor_copy(out=dst, in_=ps[:, :])
                        ev += 1
                Ytr = Yt[:]

                # prefetch the next image's input ahead of this image's stores
                # in the sync-ring FIFO
                if i + 1 < IMGS:
                    Xnext = load_x(i + 1)

                # pass 2: STG[p, (s*3+m)*258 + kh] = O_s[kw = 86m + p, kh]
                STG = spool.tile([86, 12 * OUT], F32, tag="STG")
                for g in range(2):
                    for f in range(2):
                        s = f + 2 * g
                        for m in range(3):
                            ps2 = pspool.tile([86, OUT], F32, tag="ps2", bufs=PS2_BUFS)
                            for j, q in enumerate((m, m + 1)):
                                nc.tensor.matmul(
                                    ps2[:, :],
                                    lhsT=Wr[:, (g * 4 + q) * OUT + m * 86:
                                            (g * 4 + q) * OUT + (m + 1) * 86],
                                    rhs=Ytr[:, (f * 4 + q) * OUT: (f * 4 + q + 1) * OUT],
                                    start=(j == 0),
                                    stop=(j == 1),
                                )
                            dst = STG[:, (s * 3 + m) * OUT: (s * 3 + m + 1) * OUT]
                            if ev % 2 == 0:
                                nc.scalar.copy(out=dst, in_=ps2[:, :])
                            else:
                                nc.vector.tensor_copy(out=dst, in_=ps2[:, :])
                            ev += 1

                for s in range(4):
                    nc.sync.dma_start(
                        out=y_out[i, s].rearrange("(m p) k -> p m k", p=86),
                        in_=STG[:, s * 3 * OUT:(s + 1) * 3 * OUT].rearrange(
                            "p (m k) -> p m k", m=3),
                    )
    nc.finalize()
    return nc


def _get_module() -> bass.Bass:
    global _MODULE
    if _MODULE is None:
        _MODULE = _build_module()
    return _MODULE


def make_in_maps(x: np.ndarray) -> list[dict]:
    x = np.asarray(x, dtype=np.float32)
    B, C, Hx, Wx = x.shape
    assert (Hx, Wx) == (H, W) and B * C == N_CORES * IMGS
    imgs = x.reshape(B * C, H, W)
    return [
        {"x": _round_tf32(imgs[k * IMGS:(k + 1) * IMGS]), "w": _WEIGHTS}
        for k in range(N_CORES)
    ]


def kernel(**inputs) -> np.ndarray:
    x = np.asarray(inputs["x"], dtype=np.float32)
    B, C, Hx, Wx = x.shape

    nc = _get_module()
    in_maps = make_in_maps(x)
    res = run_bass_kernel_spmd(nc, in_maps, list(range(N_CORES))).results

    full = np.concatenate([res[k]["y"] for k in range(N_CORES)], axis=0)
    # device layout is [img, s, kw, kh] -> swap to [img, s, kh, kw]
    full = full.transpose(0, 1, 3, 2)
    return np.ascontiguousarray(full.reshape(B, 4 * C, OUT, OUT)).astype(np.float32)



# revision 9
# speedup vs baseline: 5.5462x; 5.5462x over previous
"""Trainium2 Bass kernel for 2D single-level DWT (coif1, symmetric padding).

Input  x: (4, 64, 512, 512) fp32
Output  : (4, 256, 258, 258) fp32  -- per input channel: [cA, cH, cV, cD]

Math: with R_f the banded 258x512 operator of the 1D DWT along an axis
(6-tap filter, stride 2, symmetric boundary folds), the four outputs are
    cA = R_lo X R_lo^T,  cH = R_hi X R_lo^T,
    cV = R_lo X R_hi^T,  cD = R_hi X R_hi^T.

v2 design (fp16 data path, band-windowed matmuls, 32 images per core):
  pass 1 (contract rows r):   Yt_f[c, kh] = sum_r X[r, c] R_f[kh, r]
     stationary lhsT = X chunk [r:128, c:128]; moving rhs = R_f^T slice.
     R is banded: r-chunk q only contributes to kh in window [64q, 64q+66),
     so each matmul streams only ~66 columns instead of 258.  The lo/hi
     filters are packed side by side in one PSUM bank ([f:2, kh]) so one
     matmul (one LDWEIGHTS of the X chunk) serves both filters.
  pass 2 (contract cols c):   O_s[kh, kw] = sum_c Yt_f[c, kh] R_g[kw, c]
     stationary lhsT = Yt chunk [c:128, kh:128]; moving rhs = R_g^T slice,
     g in {lo, hi} packed per matmul the same way.  kh is tiled [0,128),
     [128,256) plus a 2-row remainder handled in a packed [4, kw] tile.
  PSUM accumulation relies on the hardware per-element has_written bits:
  the first matmul into a bank uses start=True (arms lazy-zero for the
  whole bank); later matmuls in the chain use start=False and may touch a
  mix of written (accumulate) and pending (overwrite) columns.

  Output is stored fp16 as y[i, khc, p, s*258 + kw] (contiguous 2 KiB+
  per partition, 128 partitions -> DMA spreads over all 16 SDMA engines;
  the v1 layout concentrated stores on 2 engines at ~37 GB/s).
"""

import os
import sys

for _p in ("/opt/trn_rl_repo", "/opt/pypackages"):
    if _p not in sys.path:
        sys.path.append(_p)

os.environ.setdefault("JAX_COMPILATION_CACHE_DIR", "/tmp/jax_comp_cache")
os.environ.setdefault("JAX_PERSISTENT_CACHE_MIN_COMPILE_TIME_SECS", "10")

import numpy as np

import concourse.bass as bass
import concourse.bacc as bacc
import concourse.mybir as mybir
from concourse.bass_utils import run_bass_kernel_spmd
from concourse.tile import TileContext

N_CORES = 8
H = W = 512
OUT = 258  # (512 + 6 - 1) // 2
IMGS = 32  # images per core (4*64/8)
F16 = mybir.dt.float16
F32 = mybir.dt.float32

# pywt coif1 decomposition filters
DEC_LO = np.array([-0.01565572813546454, -0.0727326195128539, 0.38486484686420286,
                   0.8525720202122554, 0.3378976624578092, -0.0727326195128539])
DEC_HI = np.array([0.0727326195128539, 0.3378976624578092, -0.8525720202122554,
                   0.38486484686420286, 0.0727326195128539, -0.01565572813546454])
FLEN = 6
PAD = 4
LO_F = DEC_LO[::-1]
HI_F = DEC_HI[::-1]

# kh/kw window that r/c-chunk q contributes to (from the band structure)
WINS = [(0, 66), (64, 130), (128, 194), (192, 258)]
BSP = 194  # bank split: [0,194) x2 filters = 1552B, [194,258) x2 = 512B

# If True, split matmuls so no instruction touches a mix of
# already-written and pending-zero PSUM bytes (needed only for CoreSim;
# hardware has per-element has_written bits).
INTERP_SAFE = False


def _build_R(filt: np.ndarray, n: int = W) -> np.ndarray:
    """Banded [258, 512] operator: out[k] = sum_j filt[j] * x[sym(2k + j - PAD)]."""
    out_len = (n + FLEN - 1) // 2

    def sym(i: int) -> int:
        while i < 0 or i >= n:
            if i < 0:
                i = -i - 1
            if i >= n:
                i = 2 * n - 1 - i
        return i

    R = np.zeros((out_len, n), dtype=np.float64)
    for k in range(out_len):
        for j in range(FLEN):
            R[k, sym(2 * k + j - PAD)] += filt[j]
    return R


def _check_windows(R: np.ndarray) -> None:
    for q in range(4):
        nz = np.nonzero((R[:, 128 * q:128 * (q + 1)] != 0).any(axis=1))[0]
        assert (int(nz.min()), int(nz.max()) + 1) == WINS[q], (q, nz.min(), nz.max())


def _build_weights() -> np.ndarray:
    """Interleaved: w[p, q*516 + 2k + f] = R_f[k, 128q + p], [128, 4*516] fp16.

    The lo/hi filter pair is interleaved along the matmul stream dim so a
    single 1-D rhs AP (and a 1-D PSUM out AP at columns [2lo, 2hi)) serves
    both filters per LDWEIGHTS of the stationary data chunk.
    """
    Rs = [_build_R(LO_F), _build_R(HI_F)]
    _check_windows(Rs[0])
    _check_windows(Rs[1])
    w = np.zeros((128, 4 * 2 * OUT), dtype=np.float32)
    for q in range(4):
        blk = np.zeros((128, OUT, 2), dtype=np.float32)
        for f in range(2):
            blk[:, :, f] = Rs[f][:, 128 * q:128 * (q + 1)].T
        w[:, q * 2 * OUT:(q + 1) * 2 * OUT] = blk.reshape(128, 2 * OUT)
    return w.astype(np.float16)


_WEIGHTS = _build_weights()
_MODULE = None


def _build_module() -> bass.Bass:
    nc = bacc.Bacc("TRN2", target_bir_lowering=False, debug=False)
    x_in = nc.declare_dram_parameter("x", [IMGS, 128, 4 * W], F16, isOutput=False)
    w_in = nc.declare_dram_parameter("w", [128, 4 * 2 * OUT], F16, isOutput=False)
    # y[i, khc, p, s*258 + kw] = O_s[kh = 128*khc + p, kw] for kh < 256
    y_main = nc.declare_dram_parameter("y", [IMGS, 2, 128, 4 * OUT], F16,
                                       isOutput=True)
    # yr[j*2 + f, i*516 + g*258 + kw] = O_{f+2g}[kh = 256 + j, kw]
    y_rem = nc.declare_dram_parameter("yr", [4, IMGS * 2 * OUT], F16, isOutput=True)

    with TileContext(nc) as tc:
        with (
            tc.tile_pool(name="wpool", bufs=1) as wpool,
            tc.tile_pool(name="xpool", bufs=3) as xpool,
            tc.tile_pool(name="ypool", bufs=2) as ypool,
            tc.tile_pool(name="spool", bufs=4) as spool,
            tc.tile_pool(name="rpool", bufs=1) as rpool,
            tc.tile_pool(name="psum", bufs=2, space="PSUM") as pspool,
        ):
            Wt = wpool.tile([128, 4 * 2 * OUT], F16)
            nc.sync.dma_start(out=Wt[:], in_=w_in[:])
            Wr = Wt[:]

            Crem = rpool.tile([4, IMGS * 2 * OUT], F16)
            Cv = Crem[:].rearrange("p (i g k) -> p i g k", i=IMGS, g=2)

            # Tiny PE op consuming the weight DMA so later matmuls depend
            # on it via PE program order.
            warm = pspool.tile([1, 2 * (OUT - BSP)], F32, tag="p1b", bufs=2)
            nc.tensor.matmul(warm[:, 0:1], lhsT=Wr[:, 0:1], rhs=Wr[:, 0:1],
                             start=True, stop=True)

            def load_x(i):
                X = xpool.tile([128, 4 * W], F16, tag="X", name=f"X_{i}")
                nc.sync.dma_start(out=X[:], in_=x_in[i])
                return X

            # GPSIMD can't read PSUM; balance scalar (1.2 GHz) vs vector
            # (0.96 GHz) by element count.
            def copy(dst, src, eng):
                if eng == "s":
                    nc.scalar.copy(out=dst, in_=src)
                else:
                    nc.vector.tensor_copy(out=dst, in_=src)

            def chain(lhsT_fn, A, B):
                """Accumulate the banded, filter-interleaved product into
                PSUM A [p, 388] (cols 2*kh+f, kh<194) and B [p, 128]
                (cols 2*(kh-194)+f) over contraction chunks q."""
                # (q, lo, hi, which-tile, start, stop) in kh units
                if INTERP_SAFE:
                    segs = [(0, 0, 66, 0, True, False),
                            (1, 64, 66, 0, False, False),
                            (1, 66, 130, 0, False, False),
                            (2, 128, 130, 0, False, False),
                            (2, 130, 194, 0, False, False),
                            (3, 192, 194, 0, False, True),
                            (3, 194, 258, 1, True, True)]
                else:
                    segs = [(0, 0, 66, 0, True, False),
                            (1, 64, 130, 0, False, False),
                            (2, 128, 194, 0, False, False),
                            (3, 192, 194, 0, False, True),
                            (3, 194, 258, 1, True, True)]
                for q, lo, hi, t, st, sp in segs:
                    if t == 0:
                        out = A[:, 2 * lo:2 * hi]
                    else:
                        out = B[:, 2 * (lo - BSP):2 * (hi - BSP)]
                    rhs = Wr[:, q * 2 * OUT + 2 * lo:q * 2 * OUT + 2 * hi]
                    nc.tensor.matmul(out, lhsT=lhsT_fn(q), rhs=rhs,
                                     start=st, stop=sp)

            Xnext = load_x(0)
            for i in range(IMGS):
                Xr = Xnext[:]
                Xv = Xr.rearrange("p (q c) -> p q c", q=4)

                # ---- pass 1: Yt[p, (f*4+cc)*258 + kh] = Yt_f[c=128cc+p, kh]
                Yt = ypool.tile([128, 8 * OUT], F16, tag="Yt")
                Yv = Yt[:].rearrange("p (f cc k) -> p f cc k", f=2, cc=4)
                # compacted lhsT strip for the pass-2 remainder:
                # YtR[p, cc*4 + j*2 + f] = Yt_f[c=128cc+p, 256+j]
                YtR = ypool.tile([128, 16], F16, tag="YtR")
                for cc in range(4):
                    A = pspool.tile([128, 2 * BSP], F32, tag="p1a", bufs=2)
                    B = pspool.tile([128, 2 * (OUT - BSP)], F32, tag="p1b", bufs=2)
                    A3 = A[:].rearrange("p (k f) -> p f k", f=2)
                    B3 = B[:].rearrange("p (k f) -> p f k", f=2)
                    chain(lambda q: Xv[:, q, cc * 128:(cc + 1) * 128], A[:], B[:])
                    copy(Yv[:, :, cc, 0:BSP], A3[:, :, :], "s")
                    copy(Yv[:, :, cc, BSP:OUT], B3[:, :, :], "v")
                    # last 2 kh cols (interleaved (j,f) order) for the rem chain
                    copy(YtR[:, cc * 4:(cc + 1) * 4], B[:, 124:128], "v")

                # prefetch next image ahead of this image's stores
                if i + 1 < IMGS:
                    Xnext = load_x(i + 1)

                # ---- pass 2: O_s[kh, kw], kh tiled 128+128, then 2-row rem
                for khc in range(2):
                    STG = spool.tile([128, 4 * OUT], F16, tag="STG")
                    Sv = STG[:].rearrange("p (g f2 k) -> p g f2 k", g=2, f2=2)
                    for f in range(2):
                        A = pspool.tile([128, 2 * BSP], F32, tag="p2a", bufs=2)
                        B = pspool.tile([128, 2 * (OUT - BSP)], F32, tag="p2b",
                                        bufs=2)
                        A3 = A[:].rearrange("p (k g) -> p g k", g=2)
                        B3 = B[:].rearrange("p (k g) -> p g k", g=2)
                        chain(lambda q: Yv[:, f, q, 128 * khc:128 * (khc + 1)],
                              A[:], B[:])
                        copy(Sv[:, :, f, 0:BSP], A3[:, :, :],
                             "s" if f == 0 else "v")
                        copy(Sv[:, :, f, BSP:OUT], B3[:, :, :], "v")
                    nc.sync.dma_start(out=y_main[i, khc], in_=STG[:])

                # ---- pass 2 remainder: kh in {256, 257}, psum rows j*2+f
                Ar = pspool.tile([4, 2 * BSP], F32, tag="p2a", bufs=2)
                Br = pspool.tile([4, 2 * (OUT - BSP)], F32, tag="p2b", bufs=2)
                Ar3 = Ar[:].rearrange("p (k g) -> p g k", g=2)
                Br3 = Br[:].rearrange("p (k g) -> p g k", g=2)
                chain(lambda q: YtR[:, q * 4:(q + 1) * 4], Ar[:], Br[:])
                copy(Cv[:, i, :, 0:BSP], Ar3[:, :, :], "v")
                copy(Cv[:, i, :, BSP:OUT], Br3[:, :, :], "s")

            nc.sync.dma_start(out=y_rem[:], in_=Crem[:])
    nc.finalize()
    return nc


def _get_module() -> bass.Bass:
    global _MODULE
    if _MODULE is None:
        _MODULE = _build_module()
    return _MODULE


def make_in_maps(x: np.ndarray) -> list[dict]:
    x = np.asarray(x, dtype=np.float32)
    B, C, Hx, Wx = x.shape
    assert (Hx, Wx) == (H, W) and B * C == N_CORES * IMGS
    imgs = x.reshape(B * C, H, W)
    maps = []
    for k in range(N_CORES):
        # X[p, q*512 + c] = x[i, 128q + p, c]
        xc = imgs[k * IMGS:(k + 1) * IMGS].reshape(IMGS, 4, 128, W)
        xc = np.ascontiguousarray(xc.transpose(0, 2, 1, 3)).reshape(IMGS, 128, 4 * W)
        maps.append({"x": xc.astype(np.float16), "w": _WEIGHTS})
    return maps


def kernel(**inputs) -> np.ndarray:
    x = np.asarray(inputs["x"], dtype=np.float32)
    B, C, Hx, Wx = x.shape

    nc = _get_module()
    in_maps = make_in_maps(x)
    res = run_bass_kernel_spmd(nc, in_maps, list(range(N_CORES))).results

    full = np.empty((N_CORES * IMGS, 4, OUT, OUT), dtype=np.float32)
    for k in range(N_CORES):
        ym = res[k]["y"].reshape(IMGS, 2, 128, 4, OUT)  # [i, khc, p, s, kw]
        yr = res[k]["yr"].reshape(4, IMGS, 2, OUT)      # [j*2+f, i, g, kw]
        dst = full[k * IMGS:(k + 1) * IMGS]
        dst[:, :, :256, :] = ym.transpose(0, 3, 1, 2, 4).reshape(IMGS, 4, 256, OUT)
        for f in range(2):
            for j in range(2):
                for g in range(2):
                    dst[:, f + 2 * g, 256 + j, :] = yr[j * 2 + f, :, g, :]

    return np.ascontiguousarray(full.reshape(B, 4 * C, OUT, OUT))


# revision 10
# speedup vs baseline: 6.0880x; 1.0977x over previous
"""Trainium2 Bass kernel for 2D single-level DWT (coif1, symmetric padding).

Input  x: (4, 64, 512, 512) fp32
Output  : (4, 256, 258, 258) fp32  -- per input channel: [cA, cH, cV, cD]

Math: with R_f the banded 258x512 operator of the 1D DWT along an axis
(6-tap filter, stride 2, symmetric boundary folds), the four outputs are
    cA = R_lo X R_lo^T,  cH = R_hi X R_lo^T,
    cV = R_lo X R_hi^T,  cD = R_hi X R_hi^T.

v3 design (fp16 data path, band-windowed matmuls, 32 images per core):
  pass 1 (contract rows r):   Yt_f[c, kh] = sum_r X[r, c] R_f[kh, r]
     stationary lhsT = X chunk [r:128, c:128]; moving rhs = R^T slice with
     the lo/hi filter pair interleaved along the stream dim (col 2*kh+f),
     so one matmul serves both filters per LDWEIGHTS.  R is banded:
     r-chunk q only reaches kh in [64q, 64q+66), so each matmul streams
     ~132 interleaved columns instead of 516.
  pass 2 (contract cols c):   O_s[kh, kw] = sum_c Yt_f[c, kh] R_g[kw, c]
     stationary lhsT = Yt chunk (stride-2 slice of the interleaved Yt);
     kh tiled [0,128), [128,256), plus a 2-row remainder whose lhsT is the
     4 contiguous tail columns of each Yt block.
  PSUM accumulation relies on per-element has_written bits: first matmul
  into a bank uses start=True (arms lazy-zero for the whole bank); later
  chain matmuls use start=False and may touch a mix of written
  (accumulate) and pending-zero (overwrite) columns.
  PSUM pool: two shared tags (388-col / 128-col) x 4 banks each, so a
  chain never waits on the drain of the immediately preceding chain.
  DMA: one load per 4 images, one store per 4 images, all transfers
  128-partition with >=2KiB contiguous per partition (16 SDMA engines).
"""

import os
import sys

for _p in ("/opt/trn_rl_repo", "/opt/pypackages"):
    if _p not in sys.path:
        sys.path.append(_p)

os.environ.setdefault("JAX_COMPILATION_CACHE_DIR", "/tmp/jax_comp_cache")
os.environ.setdefault("JAX_PERSISTENT_CACHE_MIN_COMPILE_TIME_SECS", "10")

import numpy as np

import concourse.bass as bass
import concourse.bacc as bacc
import concourse.mybir as mybir
from concourse.bass_utils import run_bass_kernel_spmd
from concourse.tile import TileContext

N_CORES = 8
H = W = 512
OUT = 258  # (512 + 6 - 1) // 2
IMGS = 32  # images per core (4*64/8)
BATCH = 4  # images per DMA transfer
NB = IMGS // BATCH
F16 = mybir.dt.float16
F32 = mybir.dt.float32

# pywt coif1 decomposition filters
DEC_LO = np.array([-0.01565572813546454, -0.0727326195128539, 0.38486484686420286,
                   0.8525720202122554, 0.3378976624578092, -0.0727326195128539])
DEC_HI = np.array([0.0727326195128539, 0.3378976624578092, -0.8525720202122554,
                   0.38486484686420286, 0.0727326195128539, -0.01565572813546454])
FLEN = 6
PAD = 4
LO_F = DEC_LO[::-1]
HI_F = DEC_HI[::-1]

# kh/kw window that r/c-chunk q contributes to (from the band structure)
WINS = [(0, 66), (64, 130), (128, 194), (192, 258)]
BSP = 194  # PSUM bank split: [0,194)x2 = 1552B in tile A, [194,258)x2 in B

# If True, split matmuls so no instruction touches a mix of
# already-written and pending-zero PSUM bytes (needed only for CoreSim;
# hardware has per-element has_written bits).
INTERP_SAFE = False


def _build_R(filt: np.ndarray, n: int = W) -> np.ndarray:
    """Banded [258, 512] operator: out[k] = sum_j filt[j] * x[sym(2k + j - PAD)]."""
    out_len = (n + FLEN - 1) // 2

    def sym(i: int) -> int:
        while i < 0 or i >= n:
            if i < 0:
                i = -i - 1
            if i >= n:
                i = 2 * n - 1 - i
        return i

    R = np.zeros((out_len, n), dtype=np.float64)
    for k in range(out_len):
        for j in range(FLEN):
            R[k, sym(2 * k + j - PAD)] += filt[j]
    return R


def _check_windows(R: np.ndarray) -> None:
    for q in range(4):
        nz = np.nonzero((R[:, 128 * q:128 * (q + 1)] != 0).any(axis=1))[0]
        assert (int(nz.min()), int(nz.max()) + 1) == WINS[q], (q, nz.min(), nz.max())


def _build_weights() -> np.ndarray:
    """Interleaved: w[p, q*516 + 2k + f] = R_f[k, 128q + p], [128, 4*516] fp16."""
    Rs = [_build_R(LO_F), _build_R(HI_F)]
    _check_windows(Rs[0])
    _check_windows(Rs[1])
    w = np.zeros((128, 4 * 2 * OUT), dtype=np.float32)
    for q in range(4):
        blk = np.zeros((128, OUT, 2), dtype=np.float32)
        for f in range(2):
            blk[:, :, f] = Rs[f][:, 128 * q:128 * (q + 1)].T
        w[:, q * 2 * OUT:(q + 1) * 2 * OUT] = blk.reshape(128, 2 * OUT)
    return w.astype(np.float16)


_WEIGHTS = _build_weights()
_MODULE = None


def _build_module() -> bass.Bass:
    nc = bacc.Bacc("TRN2", target_bir_lowering=False, debug=False)
    x_in = nc.declare_dram_parameter("x", [NB, 128, BATCH * 4 * W], F16,
                                     isOutput=False)
    w_in = nc.declare_dram_parameter("w", [128, 4 * 2 * OUT], F16, isOutput=False)
    # y[b, p, ((ib*2 + khc)*2 + f)*516 + 2*kw + g] = O_{f+2g}[128*khc + p, kw]
    y_main = nc.declare_dram_parameter("y", [NB, 128, BATCH * 4 * 516], F16,
                                       isOutput=True)
    # yr[j*2 + f, i*516 + 2*kw + g] = O_{f+2g}[256 + j, kw]
    y_rem = nc.declare_dram_parameter("yr", [4, IMGS * 516], F16, isOutput=True)

    with TileContext(nc) as tc:
        with (
            tc.tile_pool(name="wpool", bufs=1) as wpool,
            tc.tile_pool(name="xpool", bufs=2) as xpool,
            tc.tile_pool(name="ypool", bufs=2) as ypool,
            tc.tile_pool(name="spool", bufs=2) as spool,
            tc.tile_pool(name="rpool", bufs=1) as rpool,
            tc.tile_pool(name="psum", bufs=4, space="PSUM") as pspool,
        ):
            Wt = wpool.tile([128, 4 * 2 * OUT], F16)
            nc.sync.dma_start(out=Wt[:], in_=w_in[:])
            Wr = Wt[:]

            Crem = rpool.tile([4, IMGS * 516], F16)

            # Tiny PE op consuming the weight DMA so later matmuls depend
            # on it via PE program order.
            warm = pspool.tile([1, 2 * (OUT - BSP)], F32, tag="pB", bufs=4)
            nc.tensor.matmul(warm[:, 0:1], lhsT=Wr[:, 0:1], rhs=Wr[:, 0:1],
                             start=True, stop=True)

            def load_x(b):
                X = xpool.tile([128, BATCH * 4 * W], F16, tag="X", name=f"X_{b}")
                nc.sync.dma_start(out=X[:], in_=x_in[b])
                return X

            # GPSIMD can't read PSUM; balance scalar (1.2 GHz) vs vector
            # (0.96 GHz) by element count.
            def copy(dst, src, eng):
                if eng == "s":
                    nc.scalar.copy(out=dst, in_=src)
                else:
                    nc.vector.tensor_copy(out=dst, in_=src)

            def chain(lhsT_fn, A, B):
                """Accumulate the banded, filter-interleaved product into
                PSUM A [p, 388] (cols 2*kh+f, kh<194) and B [p, 128]
                (cols 2*(kh-194)+f) over contraction chunks q."""
                if INTERP_SAFE:
                    segs = [(0, 0, 66, 0, True, False),
                            (1, 64, 66, 0, False, False),
                            (1, 66, 130, 0, False, False),
                            (2, 128, 130, 0, False, False),
                            (2, 130, 194, 0, False, False),
                            (3, 192, 194, 0, False, True),
                            (3, 194, 258, 1, True, True)]
                else:
                    segs = [(0, 0, 66, 0, True, False),
                            (1, 64, 130, 0, False, False),
                            (2, 128, 194, 0, False, False),
                            (3, 192, 194, 0, False, True),
                            (3, 194, 258, 1, True, True)]
                for q, lo, hi, t, st, sp in segs:
                    if t == 0:
                        out = A[:, 2 * lo:2 * hi]
                    else:
                        out = B[:, 2 * (lo - BSP):2 * (hi - BSP)]
                    rhs = Wr[:, q * 2 * OUT + 2 * lo:q * 2 * OUT + 2 * hi]
                    nc.tensor.matmul(out, lhsT=lhsT_fn(q), rhs=rhs,
                                     start=st, stop=sp)

            Xcur = load_x(0)
            for b in range(NB):
                Xv = Xcur[:].rearrange("p (i q c) -> p i q c", i=BATCH, q=4)
                STG = spool.tile([128, BATCH * 4 * 516], F16, tag="STG")
                for ib in range(BATCH):
                    i = b * BATCH + ib

                    # pass 1: Yt[p, cc*516 + 2*kh + f] = Yt_f[c=128cc+p, kh]
                    Yt = ypool.tile([128, 4 * 516], F16, tag="Yt")
                    Ytr = Yt[:]
                    Ytv = Ytr.rearrange("p (cc k f) -> p cc k f", cc=4, f=2)
                    for cc in range(4):
                        A = pspool.tile([128, 2 * BSP], F32, tag="pA", bufs=4)
                        B = pspool.tile([128, 2 * (OUT - BSP)], F32, tag="pB",
                                        bufs=4)
                        chain(lambda q: Xv[:, ib, q, cc * 128:(cc + 1) * 128],
                              A[:], B[:])
                        copy(Ytr[:, cc * 516:cc * 516 + 2 * BSP], A[:, :], "s")
                        copy(Ytr[:, cc * 516 + 2 * BSP:(cc + 1) * 516],
                             B[:, :], "v")

                    # prefetch next batch ahead of this batch's store
                    if ib == 0 and b + 1 < NB:
                        Xcur = load_x(b + 1)

                    # pass 2: O_s[kh, kw], kh tiled 128+128, then 2-row rem
                    for khc in range(2):
                        for f in range(2):
                            A = pspool.tile([128, 2 * BSP], F32, tag="pA",
                                            bufs=4)
                            B = pspool.tile([128, 2 * (OUT - BSP)], F32,
                                            tag="pB", bufs=4)
                            chain(lambda q: Ytv[:, q,
                                               128 * khc:128 * (khc + 1), f],
                                  A[:], B[:])
                            off = ((ib * 2 + khc) * 2 + f) * 516
                            copy(STG[:, off:off + 2 * BSP], A[:, :],
                                 "s" if f == 0 else "v")
                            copy(STG[:, off + 2 * BSP:off + 516], B[:, :], "v")

                    # remainder rows kh in {256,257}: lhsT = 4 contiguous
                    # tail cols of each Yt block; psum rows j*2+f
                    Ar = pspool.tile([4, 2 * BSP], F32, tag="pA", bufs=4)
                    Br = pspool.tile([4, 2 * (OUT - BSP)], F32, tag="pB", bufs=4)
                    chain(lambda q: Ytr[:, q * 516 + 512:(q + 1) * 516],
                          Ar[:], Br[:])
                    copy(Crem[:, i * 516:i * 516 + 2 * BSP], Ar[:, :], "v")
                    copy(Crem[:, i * 516 + 2 * BSP:(i + 1) * 516], Br[:, :], "s")

                nc.sync.dma_start(out=y_main[b], in_=STG[:])

            nc.sync.dma_start(out=y_rem[:], in_=Crem[:])
    nc.finalize()
    return nc


def _get_module() -> bass.Bass:
    global _MODULE
    if _MODULE is None:
        _MODULE = _build_module()
    return _MODULE


def make_in_maps(x: np.ndarray) -> list[dict]:
    x = np.asarray(x, dtype=np.float32)
    B, C, Hx, Wx = x.shape
    assert (Hx, Wx) == (H, W) and B * C == N_CORES * IMGS
    imgs = x.reshape(B * C, H, W)
    maps = []
    for k in range(N_CORES):
        # X[b][p, ib*2048 + q*512 + c] = x[b*BATCH+ib, 128q + p, c]
        xc = imgs[k * IMGS:(k + 1) * IMGS].reshape(NB, BATCH, 4, 128, W)
        xc = np.ascontiguousarray(xc.transpose(0, 3, 1, 2, 4))
        maps.append({"x": xc.reshape(NB, 128, BATCH * 4 * W).astype(np.float16),
                     "w": _WEIGHTS})
    return maps


def kernel(**inputs) -> np.ndarray:
    x = np.asarray(inputs["x"], dtype=np.float32)
    B, C, Hx, Wx = x.shape

    nc = _get_module()
    in_maps = make_in_maps(x)
    res = run_bass_kernel_spmd(nc, in_maps, list(range(N_CORES))).results

    full = np.empty((N_CORES * IMGS, 4, OUT, OUT), dtype=np.float32)
    for k in range(N_CORES):
        # [b, p, ib, khc, f, kw, g]
        ym = res[k]["y"].reshape(NB, 128, BATCH, 2, 2, OUT, 2)
        yr = res[k]["yr"].reshape(4, IMGS, OUT, 2)  # [j*2+f, i, kw, g]
        dst = full[k * IMGS:(k + 1) * IMGS]
        # dst[b*BATCH+ib, f+2g, khc*128+p, kw] = ym[b, p, ib, khc, f, kw, g]
        t = ym.transpose(0, 2, 4, 6, 3, 1, 5).reshape(IMGS, 4, 256, OUT)
        # s index of t's dim-1 is f*2+g; reorder to s = f+2g -> [0,2,1,3]
        dst[:, :, :256, :] = t[:, [0, 2, 1, 3]]
        for f in range(2):
            for j in range(2):
                for g in range(2):
                    dst[:, f + 2 * g, 256 + j, :] = yr[j * 2 + f, :, :, g]

    return np.ascontiguousarray(full.reshape(B, 4 * C, OUT, OUT))


# revision 11
# speedup vs baseline: 6.9261x; 1.1377x over previous
"""Trainium2 Bass kernel for 2D single-level DWT (coif1, symmetric padding).

Input  x: (4, 64, 512, 512) fp32
Output  : (4, 256, 258, 258) fp32  -- per input channel: [cA, cH, cV, cD]

Math: with R_f the banded 258x512 operator of the 1D DWT along an axis
(6-tap filter, stride 2, symmetric boundary folds), the four outputs are
    cA = R_lo X R_lo^T,  cH = R_hi X R_lo^T,
    cV = R_lo X R_hi^T,  cD = R_hi X R_hi^T.

v4 design (fp16 data path, band-windowed matmuls, 32 images per core):
  pass 1 (contract rows r):   Yt_f[c, kh] = sum_r X[r, c] R_f[kh, r]
     stationary lhsT = X chunk [r:128, c:128]; moving rhs = R^T slice with
     the lo/hi filter pair interleaved along the stream dim (col 2*kh+f),
     so one matmul serves both filters per LDWEIGHTS.  R is banded:
     r-chunk q only reaches kh in [64q, 64q+66), so each matmul streams
     ~132 interleaved columns instead of 516.
  pass 2 (contract cols c):   O_s[kh, kw] = sum_c Yt_f[c, kh] R_g[kw, c]
     stationary lhsT = Yt chunk (stride-2 slice of the interleaved Yt);
     kh tiled [0,128), [128,256), plus a 2-row remainder whose lhsT is the
     4 contiguous tail columns of each Yt block.
  PSUM accumulation relies on per-element has_written bits: first matmul
  into a bank uses start=True (arms lazy-zero for the whole bank); later
  chain matmuls use start=False and may touch a mix of written
  (accumulate) and pending-zero (overwrite) columns.
  Chains are PAIRED into double-width PSUM tiles (A-pair spans 2 banks,
  B-pair shares 1 bank) so one engine copy drains two chains -- the
  scalar engine pays ~200ns fixed cost per instruction, so fewer, bigger
  drains matter.  The PE runs pass1(i+1) before pass2(i) so drains always
  trail a full chain-group behind the producer (no PSUM-recycle stalls).
  DMA: 2-image granularity, 128 partitions x >=2KiB contiguous per
  partition per transfer (spreads over all 16 SDMA engines).
"""

import os
import sys

for _p in ("/opt/trn_rl_repo", "/opt/pypackages"):
    if _p not in sys.path:
        sys.path.append(_p)

os.environ.setdefault("JAX_COMPILATION_CACHE_DIR", "/tmp/jax_comp_cache")
os.environ.setdefault("JAX_PERSISTENT_CACHE_MIN_COMPILE_TIME_SECS", "10")

import numpy as np

import concourse.bass as bass
import concourse.bacc as bacc
import concourse.mybir as mybir
from concourse.bass_utils import run_bass_kernel_spmd
from concourse.tile import TileContext

N_CORES = 8
H = W = 512
OUT = 258  # (512 + 6 - 1) // 2
IMGS = 32  # images per core (4*64/8)
GRP = 2    # images per DMA transfer
NG = IMGS // GRP
F16 = mybir.dt.float16
F32 = mybir.dt.float32

# pywt coif1 decomposition filters
DEC_LO = np.array([-0.01565572813546454, -0.0727326195128539, 0.38486484686420286,
                   0.8525720202122554, 0.3378976624578092, -0.0727326195128539])
DEC_HI = np.array([0.0727326195128539, 0.3378976624578092, -0.8525720202122554,
                   0.38486484686420286, 0.0727326195128539, -0.01565572813546454])
FLEN = 6
PAD = 4
LO_F = DEC_LO[::-1]
HI_F = DEC_HI[::-1]

# kh/kw window that r/c-chunk q contributes to (from the band structure)
WINS = [(0, 66), (64, 130), (128, 194), (192, 258)]
BSP = 194  # per-chain PSUM split: [0,194)x2 = 1552B (A), [194,258)x2 = 512B (B)

# If True, split matmuls so no instruction touches a mix of
# already-written and pending-zero PSUM bytes (needed only for CoreSim;
# hardware has per-element has_written bits).
INTERP_SAFE = False


def _build_R(filt: np.ndarray, n: int = W) -> np.ndarray:
    """Banded [258, 512] operator: out[k] = sum_j filt[j] * x[sym(2k + j - PAD)]."""
    out_len = (n + FLEN - 1) // 2

    def sym(i: int) -> int:
        while i < 0 or i >= n:
            if i < 0:
                i = -i - 1
            if i >= n:
                i = 2 * n - 1 - i
        return i

    R = np.zeros((out_len, n), dtype=np.float64)
    for k in range(out_len):
        for j in range(FLEN):
            R[k, sym(2 * k + j - PAD)] += filt[j]
    return R


def _check_windows(R: np.ndarray) -> None:
    for q in range(4):
        nz = np.nonzero((R[:, 128 * q:128 * (q + 1)] != 0).any(axis=1))[0]
        assert (int(nz.min()), int(nz.max()) + 1) == WINS[q], (q, nz.min(), nz.max())


def _build_weights() -> np.ndarray:
    """Interleaved: w[p, q*516 + 2k + f] = R_f[k, 128q + p], [128, 4*516] fp16."""
    Rs = [_build_R(LO_F), _build_R(HI_F)]
    _check_windows(Rs[0])
    _check_windows(Rs[1])
    w = np.zeros((128, 4 * 2 * OUT), dtype=np.float32)
    for q in range(4):
        blk = np.zeros((128, OUT, 2), dtype=np.float32)
        for f in range(2):
            blk[:, :, f] = Rs[f][:, 128 * q:128 * (q + 1)].T
        w[:, q * 2 * OUT:(q + 1) * 2 * OUT] = blk.reshape(128, 2 * OUT)
    return w.astype(np.float16)


_WEIGHTS = _build_weights()
_MODULE = None


def _build_module() -> bass.Bass:
    nc = bacc.Bacc("TRN2", target_bir_lowering=False, debug=False)
    x_in = nc.declare_dram_parameter("x", [NG, 128, GRP * 4 * W], F16,
                                     isOutput=False)
    w_in = nc.declare_dram_parameter("w", [128, 4 * 2 * OUT], F16, isOutput=False)
    # y[g, p, ((ig*2 + khc)*2 + f)*516 + 2*kw + gg] = O_{f+2gg}[128*khc + p, kw]
    y_main = nc.declare_dram_parameter("y", [NG, 128, GRP * 4 * 516], F16,
                                       isOutput=True)
    # yr[j*2 + f, i*516 + 2*kw + gg] = O_{f+2gg}[256 + j, kw]
    y_rem = nc.declare_dram_parameter("yr", [4, IMGS * 516], F16, isOutput=True)

    with TileContext(nc) as tc:
        with (
            tc.tile_pool(name="wpool", bufs=1) as wpool,
            tc.tile_pool(name="xpool", bufs=2) as xpool,
            tc.tile_pool(name="ypool", bufs=2) as ypool,
            tc.tile_pool(name="spool", bufs=2) as spool,
            tc.tile_pool(name="rpool", bufs=1) as rpool,
            tc.tile_pool(name="psum", bufs=2, space="PSUM") as pspool,
        ):
            Wt = wpool.tile([128, 4 * 2 * OUT], F16)
            nc.sync.dma_start(out=Wt[:], in_=w_in[:])
            Wr = Wt[:]

            Crem = rpool.tile([4, IMGS * 516], F16)

            # Tiny PE op consuming the weight DMA so later matmuls depend
            # on it via PE program order.
            warm = pspool.tile([1, 128], F32, tag="prB", bufs=1)
            nc.tensor.matmul(warm[:, 0:1], lhsT=Wr[:, 0:1], rhs=Wr[:, 0:1],
                             start=True, stop=True)

            def load_x(g):
                X = xpool.tile([128, GRP * 4 * W], F16, tag="X", name=f"X_{g}")
                nc.sync.dma_start(out=X[:], in_=x_in[g])
                return X

            def copy(dst, src, eng):
                if eng == "s":
                    nc.scalar.copy(out=dst, in_=src)
                else:
                    nc.vector.tensor_copy(out=dst, in_=src)

            def chain(lhsT_fn, A, B, ha, hb):
                """One banded, filter-interleaved accumulation chain into
                half `ha` of A-pair tile A (512-elem halves = bank-aligned)
                and half `hb` of B-pair tile B (128-elem halves)."""
                if INTERP_SAFE:
                    segs = [(0, 0, 66, 0, True, False),
                            (1, 64, 66, 0, False, False),
                            (1, 66, 130, 0, False, False),
                            (2, 128, 130, 0, False, False),
                            (2, 130, 194, 0, False, False),
                            (3, 192, 194, 0, False, True),
                            (3, 194, 258, 1, True, True)]
                else:
                    segs = [(0, 0, 66, 0, True, False),
                            (1, 64, 130, 0, False, False),
                            (2, 128, 194, 0, False, False),
                            (3, 192, 194, 0, False, True),
                            (3, 194, 258, 1, True, True)]
                for q, lo, hi, t, st, sp in segs:
                    if t == 0:
                        out = A[:, ha * 512 + 2 * lo:ha * 512 + 2 * hi]
                    else:
                        out = B[:, hb * 128 + 2 * (lo - BSP):
                                hb * 128 + 2 * (hi - BSP)]
                    rhs = Wr[:, q * 2 * OUT + 2 * lo:q * 2 * OUT + 2 * hi]
                    nc.tensor.matmul(out, lhsT=lhsT_fn(q), rhs=rhs,
                                     start=st, stop=sp)

            def pair_views(A, B):
                Ah = A[:].rearrange("p (h k) -> p h k", h=2)[:, :, 0:2 * BSP]
                Bh = B[:].rearrange("p (h k) -> p h k", h=2)
                return Ah, Bh

            def pass1(Xv, ig):
                """4 paired chains; returns the interleaved Yt tile
                Yt[p, cc*516 + 2*kh + f]."""
                Yt = ypool.tile([128, 4 * 516], F16, tag="Yt")
                Ytv = Yt[:].rearrange("p (cc k) -> p cc k", cc=4)
                for cp in range(2):  # cc pairs (0,1), (2,3)
                    A = pspool.tile([128, 1024], F32, tag="pAA", bufs=2)
                    B = pspool.tile([128, 256], F32, tag="pBB", bufs=2)
                    for h in range(2):
                        cc = cp * 2 + h
                        chain(lambda q: Xv[:, ig, q, cc * 128:(cc + 1) * 128],
                              A[:], B[:], h, h)
                    Ah, Bh = pair_views(A, B)
                    copy(Ytv[:, 2 * cp:2 * cp + 2, 0:2 * BSP], Ah, "s")
                    copy(Ytv[:, 2 * cp:2 * cp + 2, 2 * BSP:516], Bh, "v")
                return Yt

            def pass2(Yt, STG, ig, i):
                Ytr = Yt[:]
                Ytv4 = Ytr.rearrange("p (cc k f) -> p cc k f", cc=4, f=2)
                Sv = STG[:].rearrange("p (blk k) -> p blk k", k=516)
                for khc in range(2):  # pair over f
                    A = pspool.tile([128, 1024], F32, tag="pAA", bufs=2)
                    B = pspool.tile([128, 256], F32, tag="pBB", bufs=2)
                    for f in range(2):
                        chain(lambda q: Ytv4[:, q, 128 * khc:128 * (khc + 1), f],
                              A[:], B[:], f, f)
                    Ah, Bh = pair_views(A, B)
                    base = (ig * 2 + khc) * 2
                    copy(Sv[:, base:base + 2, 0:2 * BSP], Ah,
                         "s" if khc == 0 else "v")
                    copy(Sv[:, base:base + 2, 2 * BSP:516], Bh, "v")
                # remainder rows kh in {256,257}: lhsT = 4 contiguous tail
                # cols of each Yt block; psum rows j*2+f
                Ar = pspool.tile([4, 2 * BSP], F32, tag="prA", bufs=1)
                Br = pspool.tile([4, 128], F32, tag="prB", bufs=1)
                chain(lambda q: Ytr[:, q * 516 + 512:(q + 1) * 516],
                      Ar[:], Br[:], 0, 0)
                copy(Crem[:, i * 516:i * 516 + 2 * BSP], Ar[:, :], "v")
                copy(Crem[:, i * 516 + 2 * BSP:(i + 1) * 516], Br[:, :], "s")

            # software pipeline: PE runs pass1(i+1) before pass2(i)
            Xg = {0: load_x(0)}
            Xv = {0: Xg[0][:].rearrange("p (i q c) -> p i q c", i=GRP, q=4)}
            Yts = {0: None}
            Yts[0] = pass1(Xv[0], 0)
            STG = None
            for i in range(IMGS):
                g, ig = divmod(i, GRP)
                if ig == 0:
                    if g + 1 < NG:
                        Xg[g + 1] = load_x(g + 1)
                        Xv[g + 1] = Xg[g + 1][:].rearrange(
                            "p (i q c) -> p i q c", i=GRP, q=4)
                    STG = spool.tile([128, GRP * 4 * 516], F16, tag="STG")
                if i + 1 < IMGS:
                    g1, ig1 = divmod(i + 1, GRP)
                    Yts[i + 1] = pass1(Xv[g1], ig1)
                pass2(Yts[i], STG, ig, i)
                del Yts[i]
                if ig == GRP - 1:
                    nc.sync.dma_start(out=y_main[g], in_=STG[:])

            nc.sync.dma_start(out=y_rem[:], in_=Crem[:])
    nc.finalize()
    return nc


def _get_module() -> bass.Bass:
    global _MODULE
    if _MODULE is None:
        _MODULE = _build_module()
    return _MODULE


def make_in_maps(x: np.ndarray) -> list[dict]:
    x = np.asarray(x, dtype=np.float32)
    B, C, Hx, Wx = x.shape
    assert (Hx, Wx) == (H, W) and B * C == N_CORES * IMGS
    imgs = x.reshape(B * C, H, W)
    maps = []
    for k in range(N_CORES):
        # X[g][p, ig*2048 + q*512 + c] = x[g*GRP+ig, 128q + p, c]
        xc = imgs[k * IMGS:(k + 1) * IMGS].reshape(NG, GRP, 4, 128, W)
        xc = np.ascontiguousarray(xc.transpose(0, 3, 1, 2, 4))
        maps.append({"x": xc.reshape(NG, 128, GRP * 4 * W).astype(np.float16),
                     "w": _WEIGHTS})
    return maps


def kernel(**inputs) -> np.ndarray:
    x = np.asarray(inputs["x"], dtype=np.float32)
    B, C, Hx, Wx = x.shape

    nc = _get_module()
    in_maps = make_in_maps(x)
    res = run_bass_kernel_spmd(nc, in_maps, list(range(N_CORES))).results

    full = np.empty((N_CORES * IMGS, 4, OUT, OUT), dtype=np.float32)
    for k in range(N_CORES):
        # [g, p, ig, khc, f, kw, gg]
        ym = res[k]["y"].reshape(NG, 128, GRP, 2, 2, OUT, 2)
        yr = res[k]["yr"].reshape(4, IMGS, OUT, 2)  # [j*2+f, i, kw, gg]
        dst = full[k * IMGS:(k + 1) * IMGS]
        # dst[g*GRP+ig, f+2gg, khc*128+p, kw] = ym[g, p, ig, khc, f, kw, gg]
        t = ym.transpose(0, 2, 4, 6, 3, 1, 5).reshape(IMGS, 4, 256, OUT)
        # t's dim-1 is f*2+gg; reorder to s = f+2gg -> fg indices [0,2,1,3]
        dst[:, :, :256, :] = t[:, [0, 2, 1, 3]]
        for f in range(2):
            for j in range(2):
                for g in range(2):
                    dst[:, f + 2 * g, 256 + j, :] = yr[j * 2 + f, :, :, g]

    return np.ascontiguousarray(full.reshape(B, 4 * C, OUT, OUT))


# revision 12
# speedup vs baseline: 6.9815x; 1.0080x over previous
"""Trainium2 Bass kernel for 2D single-level DWT (coif1, symmetric padding).

Input  x: (4, 64, 512, 512) fp32
Output  : (4, 256, 258, 258) fp32  -- per input channel: [cA, cH, cV, cD]

Math: with R_f the banded 258x512 operator of the 1D DWT along an axis
(6-tap filter, stride 2, symmetric boundary folds), the four outputs are
    cA = R_lo X R_lo^T,  cH = R_hi X R_lo^T,
    cV = R_lo X R_hi^T,  cD = R_hi X R_hi^T.

v4 design (fp16 data path, band-windowed matmuls, 32 images per core):
  pass 1 (contract rows r):   Yt_f[c, kh] = sum_r X[r, c] R_f[kh, r]
     stationary lhsT = X chunk [r:128, c:128]; moving rhs = R^T slice with
     the lo/hi filter pair interleaved along the stream dim (col 2*kh+f),
     so one matmul serves both filters per LDWEIGHTS.  R is banded:
     r-chunk q only reaches kh in [64q, 64q+66), so each matmul streams
     ~132 interleaved columns instead of 516.
  pass 2 (contract cols c):   O_s[kh, kw] = sum_c Yt_f[c, kh] R_g[kw, c]
     stationary lhsT = Yt chunk (stride-2 slice of the interleaved Yt);
     kh tiled [0,128), [128,256), plus a 2-row remainder whose lhsT is the
     4 contiguous tail columns of each Yt block.
  PSUM accumulation relies on per-element has_written bits: first matmul
  into a bank uses start=True (arms lazy-zero for the whole bank); later
  chain matmuls use start=False and may touch a mix of written
  (accumulate) and pending-zero (overwrite) columns.
  Chains are PAIRED into double-width PSUM tiles (A-pair spans 2 banks,
  B-pair shares 1 bank) so one engine copy drains two chains -- the
  scalar engine pays ~200ns fixed cost per instruction, so fewer, bigger
  drains matter.  The PE runs pass1(i+1) before pass2(i) so drains always
  trail a full chain-group behind the producer (no PSUM-recycle stalls).
  DMA: 2-image granularity, 128 partitions x >=2KiB contiguous per
  partition per transfer (spreads over all 16 SDMA engines).
"""

import os
import sys

for _p in ("/opt/trn_rl_repo", "/opt/pypackages"):
    if _p not in sys.path:
        sys.path.append(_p)

os.environ.setdefault("JAX_COMPILATION_CACHE_DIR", "/tmp/jax_comp_cache")
os.environ.setdefault("JAX_PERSISTENT_CACHE_MIN_COMPILE_TIME_SECS", "10")

import numpy as np

import concourse.bass as bass
import concourse.bacc as bacc
import concourse.mybir as mybir
from concourse.bass_utils import run_bass_kernel_spmd
from concourse.tile import TileContext

N_CORES = 8
H = W = 512
OUT = 258  # (512 + 6 - 1) // 2
IMGS = 32  # images per core (4*64/8)
GRP = 2    # images per DMA transfer
NG = IMGS // GRP
F16 = mybir.dt.float16
F32 = mybir.dt.float32

# pywt coif1 decomposition filters
DEC_LO = np.array([-0.01565572813546454, -0.0727326195128539, 0.38486484686420286,
                   0.8525720202122554, 0.3378976624578092, -0.0727326195128539])
DEC_HI = np.array([0.0727326195128539, 0.3378976624578092, -0.8525720202122554,
                   0.38486484686420286, 0.0727326195128539, -0.01565572813546454])
FLEN = 6
PAD = 4
LO_F = DEC_LO[::-1]
HI_F = DEC_HI[::-1]

# kh/kw window that r/c-chunk q contributes to (from the band structure)
WINS = [(0, 66), (64, 130), (128, 194), (192, 258)]
BSP = 194  # per-chain PSUM split: [0,194)x2 = 1552B (A), [194,258)x2 = 512B (B)

# If True, split matmuls so no instruction touches a mix of
# already-written and pending-zero PSUM bytes (needed only for CoreSim;
# hardware has per-element has_written bits).
INTERP_SAFE = False


def _build_R(filt: np.ndarray, n: int = W) -> np.ndarray:
    """Banded [258, 512] operator: out[k] = sum_j filt[j] * x[sym(2k + j - PAD)]."""
    out_len = (n + FLEN - 1) // 2

    def sym(i: int) -> int:
        while i < 0 or i >= n:
            if i < 0:
                i = -i - 1
            if i >= n:
                i = 2 * n - 1 - i
        return i

    R = np.zeros((out_len, n), dtype=np.float64)
    for k in range(out_len):
        for j in range(FLEN):
            R[k, sym(2 * k + j - PAD)] += filt[j]
    return R


def _check_windows(R: np.ndarray) -> None:
    for q in range(4):
        nz = np.nonzero((R[:, 128 * q:128 * (q + 1)] != 0).any(axis=1))[0]
        assert (int(nz.min()), int(nz.max()) + 1) == WINS[q], (q, nz.min(), nz.max())


def _build_weights() -> np.ndarray:
    """Interleaved: w[p, q*516 + 2k + f] = R_f[k, 128q + p], [128, 4*516] fp16."""
    Rs = [_build_R(LO_F), _build_R(HI_F)]
    _check_windows(Rs[0])
    _check_windows(Rs[1])
    w = np.zeros((128, 4 * 2 * OUT), dtype=np.float32)
    for q in range(4):
        blk = np.zeros((128, OUT, 2), dtype=np.float32)
        for f in range(2):
            blk[:, :, f] = Rs[f][:, 128 * q:128 * (q + 1)].T
        w[:, q * 2 * OUT:(q + 1) * 2 * OUT] = blk.reshape(128, 2 * OUT)
    return w.astype(np.float16)


_WEIGHTS = _build_weights()
_MODULE = None


def _build_module() -> bass.Bass:
    nc = bacc.Bacc("TRN2", target_bir_lowering=False, debug=False)
    x_in = nc.declare_dram_parameter("x", [NG, 128, GRP * 4 * W], F16,
                                     isOutput=False)
    w_in = nc.declare_dram_parameter("w", [128, 4 * 2 * OUT], F16, isOutput=False)
    # y[g, p, ((ig*2 + khc)*2 + f)*516 + 2*kw + gg] = O_{f+2gg}[128*khc + p, kw]
    y_main = nc.declare_dram_parameter("y", [NG, 128, GRP * 4 * 516], F16,
                                       isOutput=True)
    # yr[j*2 + f, i*516 + 2*kw + gg] = O_{f+2gg}[256 + j, kw]
    y_rem = nc.declare_dram_parameter("yr", [4, IMGS * 516], F16, isOutput=True)

    with TileContext(nc) as tc:
        with (
            tc.tile_pool(name="wpool", bufs=1) as wpool,
            tc.tile_pool(name="xpool", bufs=2) as xpool,
            tc.tile_pool(name="ypool", bufs=2) as ypool,
            tc.tile_pool(name="spool", bufs=2) as spool,
            tc.tile_pool(name="rpool", bufs=1) as rpool,
            tc.tile_pool(name="psum", bufs=2, space="PSUM") as pspool,
        ):
            Wt = wpool.tile([128, 4 * 2 * OUT], F16)
            nc.sync.dma_start(out=Wt[:], in_=w_in[:])
            Wr = Wt[:]

            Crem = rpool.tile([4, IMGS * 516], F16)

            # Tiny PE op consuming the weight DMA so later matmuls depend
            # on it via PE program order.
            warm = pspool.tile([1, 256], F32, tag="pBB", bufs=2)
            nc.tensor.matmul(warm[:, 0:1], lhsT=Wr[:, 0:1], rhs=Wr[:, 0:1],
                             start=True, stop=True)

            def load_x(g):
                X = xpool.tile([128, GRP * 4 * W], F16, tag="X", name=f"X_{g}")
                nc.sync.dma_start(out=X[:], in_=x_in[g])
                return X

            def copy(dst, src, eng):
                if eng == "s":
                    nc.scalar.copy(out=dst, in_=src)
                else:
                    nc.vector.tensor_copy(out=dst, in_=src)

            def chain(lhsT_fn, A, B, ha, hb):
                """One banded, filter-interleaved accumulation chain into
                half `ha` of A-pair tile A (512-elem halves = bank-aligned)
                and half `hb` of B-pair tile B (128-elem halves)."""
                if INTERP_SAFE:
                    segs = [(0, 0, 66, 0, True, False),
                            (1, 64, 66, 0, False, False),
                            (1, 66, 130, 0, False, False),
                            (2, 128, 130, 0, False, False),
                            (2, 130, 194, 0, False, False),
                            (3, 192, 194, 0, False, True),
                            (3, 194, 258, 1, True, True)]
                else:
                    segs = [(0, 0, 66, 0, True, False),
                            (1, 64, 130, 0, False, False),
                            (2, 128, 194, 0, False, False),
                            (3, 192, 194, 0, False, True),
                            (3, 194, 258, 1, True, True)]
                for q, lo, hi, t, st, sp in segs:
                    if t == 0:
                        out = A[:, ha * 512 + 2 * lo:ha * 512 + 2 * hi]
                    else:
                        out = B[:, hb * 128 + 2 * (lo - BSP):
                                hb * 128 + 2 * (hi - BSP)]
                    rhs = Wr[:, q * 2 * OUT + 2 * lo:q * 2 * OUT + 2 * hi]
                    nc.tensor.matmul(out, lhsT=lhsT_fn(q), rhs=rhs,
                                     start=st, stop=sp)

            def pair_views(A, B):
                Ah = A[:].rearrange("p (h k) -> p h k", h=2)[:, :, 0:2 * BSP]
                Bh = B[:].rearrange("p (h k) -> p h k", h=2)
                return Ah, Bh

            def pass1(Xv, ig):
                """4 paired chains; returns the interleaved Yt tile
                Yt[p, cc*516 + 2*kh + f]."""
                Yt = ypool.tile([128, 4 * 516], F16, tag="Yt")
                Ytv = Yt[:].rearrange("p (cc k) -> p cc k", cc=4)
                for cp in range(2):  # cc pairs (0,1), (2,3)
                    A = pspool.tile([128, 1024], F32, tag="pAA", bufs=3)
                    B = pspool.tile([128, 256], F32, tag="pBB", bufs=2)
                    for h in range(2):
                        cc = cp * 2 + h
                        chain(lambda q: Xv[:, ig, q, cc * 128:(cc + 1) * 128],
                              A[:], B[:], h, h)
                    Ah, Bh = pair_views(A, B)
                    copy(Ytv[:, 2 * cp:2 * cp + 2, 0:2 * BSP], Ah, "s")
                    copy(Ytv[:, 2 * cp:2 * cp + 2, 2 * BSP:516], Bh, "v")
                return Yt

            def pass2(Yt, STG, ig, i):
                Ytr = Yt[:]
                Ytv4 = Ytr.rearrange("p (cc k f) -> p cc k f", cc=4, f=2)
                Sv = STG[:].rearrange("p (blk k) -> p blk k", k=516)
                for khc in range(2):  # pair over f
                    A = pspool.tile([128, 1024], F32, tag="pAA", bufs=3)
                    B = pspool.tile([128, 256], F32, tag="pBB", bufs=2)
                    for f in range(2):
                        chain(lambda q: Ytv4[:, q, 128 * khc:128 * (khc + 1), f],
                              A[:], B[:], f, f)
                    Ah, Bh = pair_views(A, B)
                    base = (ig * 2 + khc) * 2
                    copy(Sv[:, base:base + 2, 0:2 * BSP], Ah,
                         "s" if khc == 0 else "v")
                    copy(Sv[:, base:base + 2, 2 * BSP:516], Bh, "v")
                # remainder rows kh in {256,257}: lhsT = 4 contiguous tail
                # cols of each Yt block; psum rows j*2+f.  Uses one pAA
                # tile: A part in bank 0, B part at the start of bank 1.
                Rt = pspool.tile([4, 1024], F32, tag="pAA", bufs=3)
                chain(lambda q: Ytr[:, q * 516 + 512:(q + 1) * 516],
                      Rt[:], Rt[:], 0, 4)
                copy(Crem[:, i * 516:i * 516 + 2 * BSP], Rt[:, 0:2 * BSP], "v")
                copy(Crem[:, i * 516 + 2 * BSP:(i + 1) * 516],
                     Rt[:, 512:640], "s")

            # software pipeline: PE runs pass1(i+1) before pass2(i)
            Xg = {0: load_x(0)}
            Xv = {0: Xg[0][:].rearrange("p (i q c) -> p i q c", i=GRP, q=4)}
            Yts = {0: None}
            Yts[0] = pass1(Xv[0], 0)
            STG = None
            for i in range(IMGS):
                g, ig = divmod(i, GRP)
                if ig == 0:
                    if g + 1 < NG:
                        Xg[g + 1] = load_x(g + 1)
                        Xv[g + 1] = Xg[g + 1][:].rearrange(
                            "p (i q c) -> p i q c", i=GRP, q=4)
                    STG = spool.tile([128, GRP * 4 * 516], F16, tag="STG")
                if i + 1 < IMGS:
                    g1, ig1 = divmod(i + 1, GRP)
                    Yts[i + 1] = pass1(Xv[g1], ig1)
                pass2(Yts[i], STG, ig, i)
                del Yts[i]
                if ig == GRP - 1:
                    nc.gpsimd.dma_start(out=y_main[g], in_=STG[:])

            nc.gpsimd.dma_start(out=y_rem[:], in_=Crem[:])
    nc.finalize()
    return nc


def _get_module() -> bass.Bass:
    global _MODULE
    if _MODULE is None:
        _MODULE = _build_module()
    return _MODULE


def make_in_maps(x: np.ndarray) -> list[dict]:
    x = np.asarray(x, dtype=np.float32)
    B, C, Hx, Wx = x.shape
    assert (Hx, Wx) == (H, W) and B * C == N_CORES * IMGS
    imgs = x.reshape(B * C, H, W)
    maps = []
    for k in range(N_CORES):
        # X[g][p, ig*2048 + q*512 + c] = x[g*GRP+ig, 128q + p, c]
        xc = imgs[k * IMGS:(k + 1) * IMGS].reshape(NG, GRP, 4, 128, W)
        xc = np.ascontiguousarray(xc.transpose(0, 3, 1, 2, 4))
        maps.append({"x": xc.reshape(NG, 128, GRP * 4 * W).astype(np.float16),
                     "w": _WEIGHTS})
    return maps


def kernel(**inputs) -> np.ndarray:
    x = np.asarray(inputs["x"], dtype=np.float32)
    B, C, Hx, Wx = x.shape

    nc = _get_module()
    in_maps = make_in_maps(x)
    res = run_bass_kernel_spmd(nc, in_maps, list(range(N_CORES))).results

    full = np.empty((N_CORES * IMGS, 4, OUT, OUT), dtype=np.float32)
    for k in range(N_CORES):
        # [g, p, ig, khc, f, kw, gg]
        ym = res[k]["y"].reshape(NG, 128, GRP, 2, 2, OUT, 2)
        yr = res[k]["yr"].reshape(4, IMGS, OUT, 2)  # [j*2+f, i, kw, gg]
        dst = full[k * IMGS:(k + 1) * IMGS]
        # dst[g*GRP+ig, f+2gg, khc*128+p, kw] = ym[g, p, ig, khc, f, kw, gg]
        t = ym.transpose(0, 2, 4, 6, 3, 1, 5).reshape(IMGS, 4, 256, OUT)
        # t's dim-1 is f*2+gg; reorder to s = f+2gg -> fg indices [0,2,1,3]
        dst[:, :, :256, :] = t[:, [0, 2, 1, 3]]
        for f in range(2):
            for j in range(2):
                for g in range(2):
                    dst[:, f + 2 * g, 256 + j, :] = yr[j * 2 + f, :, :, g]

    return np.ascontiguousarray(full.reshape(B, 4 * C, OUT, OUT))


# revision 13
# speedup vs baseline: 7.1894x; 1.0298x over previous
"""Trainium2 Bass kernel for 2D single-level DWT (coif1, symmetric padding).

Input  x: (4, 64, 512, 512) fp32
Output  : (4, 256, 258, 258) fp32  -- per input channel: [cA, cH, cV, cD]

Math: with R_f the banded 258x512 operator of the 1D DWT along an axis
(6-tap filter, stride 2, symmetric boundary folds), the four outputs are
    cA = R_lo X R_lo^T,  cH = R_hi X R_lo^T,
    cV = R_lo X R_hi^T,  cD = R_hi X R_hi^T.

v4 design (fp16 data path, band-windowed matmuls, 32 images per core):
  pass 1 (contract rows r):   Yt_f[c, kh] = sum_r X[r, c] R_f[kh, r]
     stationary lhsT = X chunk [r:128, c:128]; moving rhs = R^T slice with
     the lo/hi filter pair interleaved along the stream dim (col 2*kh+f),
     so one matmul serves both filters per LDWEIGHTS.  R is banded:
     r-chunk q only reaches kh in [64q, 64q+66), so each matmul streams
     ~132 interleaved columns instead of 516.
  pass 2 (contract cols c):   O_s[kh, kw] = sum_c Yt_f[c, kh] R_g[kw, c]
     stationary lhsT = Yt chunk (stride-2 slice of the interleaved Yt);
     kh tiled [0,128), [128,256), plus a 2-row remainder whose lhsT is the
     4 contiguous tail columns of each Yt block.
  PSUM accumulation relies on per-element has_written bits: first matmul
  into a bank uses start=True (arms lazy-zero for the whole bank); later
  chain matmuls use start=False and may touch a mix of written
  (accumulate) and pending-zero (overwrite) columns.
  Chains are PAIRED into double-width PSUM tiles (A-pair spans 2 banks,
  B-pair shares 1 bank) so one engine copy drains two chains -- the
  scalar engine pays ~200ns fixed cost per instruction, so fewer, bigger
  drains matter.  The PE runs pass1(i+1) before pass2(i) so drains always
  trail a full chain-group behind the producer (no PSUM-recycle stalls).
  DMA: 2-image granularity, 128 partitions x >=2KiB contiguous per
  partition per transfer (spreads over all 16 SDMA engines).
"""

import os
import sys

for _p in ("/opt/trn_rl_repo", "/opt/pypackages"):
    if _p not in sys.path:
        sys.path.append(_p)

os.environ.setdefault("JAX_COMPILATION_CACHE_DIR", "/tmp/jax_comp_cache")
os.environ.setdefault("JAX_PERSISTENT_CACHE_MIN_COMPILE_TIME_SECS", "10")

import numpy as np

import concourse.bass as bass
import concourse.bacc as bacc
import concourse.mybir as mybir
from concourse.bass_utils import run_bass_kernel_spmd
from concourse.tile import TileContext

N_CORES = 8
H = W = 512
OUT = 258  # (512 + 6 - 1) // 2
IMGS = 32  # images per core (4*64/8)
GRP = 2    # images per DMA transfer
NG = IMGS // GRP
F16 = mybir.dt.float16
F32 = mybir.dt.float32

# pywt coif1 decomposition filters
DEC_LO = np.array([-0.01565572813546454, -0.0727326195128539, 0.38486484686420286,
                   0.8525720202122554, 0.3378976624578092, -0.0727326195128539])
DEC_HI = np.array([0.0727326195128539, 0.3378976624578092, -0.8525720202122554,
                   0.38486484686420286, 0.0727326195128539, -0.01565572813546454])
FLEN = 6
PAD = 4
LO_F = DEC_LO[::-1]
HI_F = DEC_HI[::-1]

# kh/kw window that r/c-chunk q contributes to (from the band structure)
WINS = [(0, 66), (64, 130), (128, 194), (192, 258)]
BSP = 194  # per-chain PSUM split: [0,194)x2 = 1552B (A), [194,258)x2 = 512B (B)

# If True, split matmuls so no instruction touches a mix of
# already-written and pending-zero PSUM bytes (needed only for CoreSim;
# hardware has per-element has_written bits).
INTERP_SAFE = False


def _build_R(filt: np.ndarray, n: int = W) -> np.ndarray:
    """Banded [258, 512] operator: out[k] = sum_j filt[j] * x[sym(2k + j - PAD)]."""
    out_len = (n + FLEN - 1) // 2

    def sym(i: int) -> int:
        while i < 0 or i >= n:
            if i < 0:
                i = -i - 1
            if i >= n:
                i = 2 * n - 1 - i
        return i

    R = np.zeros((out_len, n), dtype=np.float64)
    for k in range(out_len):
        for j in range(FLEN):
            R[k, sym(2 * k + j - PAD)] += filt[j]
    return R


def _check_windows(R: np.ndarray) -> None:
    for q in range(4):
        nz = np.nonzero((R[:, 128 * q:128 * (q + 1)] != 0).any(axis=1))[0]
        assert (int(nz.min()), int(nz.max()) + 1) == WINS[q], (q, nz.min(), nz.max())


def _build_weights() -> np.ndarray:
    """Interleaved: w[p, q*516 + 2k + f] = R_f[k, 128q + p], [128, 4*516] fp16."""
    Rs = [_build_R(LO_F), _build_R(HI_F)]
    _check_windows(Rs[0])
    _check_windows(Rs[1])
    w = np.zeros((128, 4 * 2 * OUT), dtype=np.float32)
    for q in range(4):
        blk = np.zeros((128, OUT, 2), dtype=np.float32)
        for f in range(2):
            blk[:, :, f] = Rs[f][:, 128 * q:128 * (q + 1)].T
        w[:, q * 2 * OUT:(q + 1) * 2 * OUT] = blk.reshape(128, 2 * OUT)
    return w.astype(np.float16)


_WEIGHTS = _build_weights()
_MODULE = None


def _build_module() -> bass.Bass:
    nc = bacc.Bacc("TRN2", target_bir_lowering=False, debug=False)
    x_in = nc.declare_dram_parameter("x", [NG, 128, GRP * 4 * W], F16,
                                     isOutput=False)
    w_in = nc.declare_dram_parameter("w", [128, 4 * 2 * OUT], F16, isOutput=False)
    # y[g, p, ((ig*2 + khc)*2 + f)*516 + 2*kw + gg] = O_{f+2gg}[128*khc + p, kw]
    y_main = nc.declare_dram_parameter("y", [NG, 128, GRP * 4 * 516], F16,
                                       isOutput=True)
    # yr[j*2 + f, i*516 + 2*kw + gg] = O_{f+2gg}[256 + j, kw]
    y_rem = nc.declare_dram_parameter("yr", [4, IMGS * 516], F16, isOutput=True)

    with TileContext(nc) as tc:
        with (
            tc.tile_pool(name="wpool", bufs=1) as wpool,
            tc.tile_pool(name="xpool", bufs=3) as xpool,
            tc.tile_pool(name="ypool", bufs=2) as ypool,
            tc.tile_pool(name="spool", bufs=2) as spool,
            tc.tile_pool(name="rpool", bufs=1) as rpool,
            tc.tile_pool(name="psum", bufs=2, space="PSUM") as pspool,
        ):
            Wt = wpool.tile([128, 4 * 2 * OUT], F16)
            Crem = rpool.tile([4, IMGS * 516], F16)

            def load_x(g):
                X = xpool.tile([128, GRP * 4 * W], F16, tag="X", name=f"X_{g}")
                nc.sync.dma_start(out=X[:], in_=x_in[g])
                return X

            X0 = load_x(0)
            nc.sync.dma_start(out=Wt[:], in_=w_in[:])
            Wr = Wt[:]

            # Tiny PE op consuming the weight DMA so later matmuls depend
            # on it via PE program order.
            warm = pspool.tile([1, 256], F32, tag="pBB", bufs=2)
            nc.tensor.matmul(warm[:, 0:1], lhsT=Wr[:, 0:1], rhs=Wr[:, 0:1],
                             start=True, stop=True)

            def copy(dst, src, eng):
                if eng == "s":
                    nc.scalar.copy(out=dst, in_=src)
                else:
                    nc.vector.tensor_copy(out=dst, in_=src)

            def chain(lhsT_fn, A, B, ha, hb):
                """One banded, filter-interleaved accumulation chain into
                half `ha` of A-pair tile A (512-elem halves = bank-aligned)
                and half `hb` of B-pair tile B (128-elem halves)."""
                if INTERP_SAFE:
                    segs = [(0, 0, 66, 0, True, False),
                            (1, 64, 66, 0, False, False),
                            (1, 66, 130, 0, False, False),
                            (2, 128, 130, 0, False, False),
                            (2, 130, 194, 0, False, False),
                            (3, 192, 194, 0, False, True),
                            (3, 194, 258, 1, True, True)]
                else:
                    segs = [(0, 0, 66, 0, True, False),
                            (1, 64, 130, 0, False, False),
                            (2, 128, 194, 0, False, False),
                            (3, 192, 194, 0, False, True),
                            (3, 194, 258, 1, True, True)]
                for q, lo, hi, t, st, sp in segs:
                    if t == 0:
                        out = A[:, ha * 512 + 2 * lo:ha * 512 + 2 * hi]
                    else:
                        out = B[:, hb * 128 + 2 * (lo - BSP):
                                hb * 128 + 2 * (hi - BSP)]
                    rhs = Wr[:, q * 2 * OUT + 2 * lo:q * 2 * OUT + 2 * hi]
                    nc.tensor.matmul(out, lhsT=lhsT_fn(q), rhs=rhs,
                                     start=st, stop=sp)

            def pair_views(A, B):
                Ah = A[:].rearrange("p (h k) -> p h k", h=2)[:, :, 0:2 * BSP]
                Bh = B[:].rearrange("p (h k) -> p h k", h=2)
                return Ah, Bh

            def pass1(Xv, ig):
                """4 paired chains; returns the interleaved Yt tile
                Yt[p, cc*516 + 2*kh + f]."""
                Yt = ypool.tile([128, 4 * 516], F16, tag="Yt")
                Ytv = Yt[:].rearrange("p (cc k) -> p cc k", cc=4)
                for cp in range(2):  # cc pairs (0,1), (2,3)
                    A = pspool.tile([128, 1024], F32, tag="pAA", bufs=3)
                    B = pspool.tile([128, 256], F32, tag="pBB", bufs=2)
                    for h in range(2):
                        cc = cp * 2 + h
                        chain(lambda q: Xv[:, ig, q, cc * 128:(cc + 1) * 128],
                              A[:], B[:], h, h)
                    Ah, Bh = pair_views(A, B)
                    copy(Ytv[:, 2 * cp:2 * cp + 2, 0:2 * BSP], Ah, "s")
                    copy(Ytv[:, 2 * cp:2 * cp + 2, 2 * BSP:516], Bh, "v")
                return Yt

            def pass2(Yt, STG, ig, i):
                Ytr = Yt[:]
                Ytv4 = Ytr.rearrange("p (cc k f) -> p cc k f", cc=4, f=2)
                Sv = STG[:].rearrange("p (blk k) -> p blk k", k=516)
                for khc in range(2):  # pair over f
                    A = pspool.tile([128, 1024], F32, tag="pAA", bufs=3)
                    B = pspool.tile([128, 256], F32, tag="pBB", bufs=2)
                    for f in range(2):
                        chain(lambda q: Ytv4[:, q, 128 * khc:128 * (khc + 1), f],
                              A[:], B[:], f, f)
                    Ah, Bh = pair_views(A, B)
                    base = (ig * 2 + khc) * 2
                    copy(Sv[:, base:base + 2, 0:2 * BSP], Ah,
                         "s" if khc == 0 else "v")
                    copy(Sv[:, base:base + 2, 2 * BSP:516], Bh, "v")
                # remainder rows kh in {256,257}: lhsT = 4 contiguous tail
                # cols of each Yt block; psum rows j*2+f.  Uses one pAA
                # tile: A part in bank 0, B part at the start of bank 1.
                Rt = pspool.tile([4, 1024], F32, tag="pAA", bufs=3)
                chain(lambda q: Ytr[:, q * 516 + 512:(q + 1) * 516],
                      Rt[:], Rt[:], 0, 4)
                copy(Crem[:, i * 516:i * 516 + 2 * BSP], Rt[:, 0:2 * BSP], "v")
                copy(Crem[:, i * 516 + 2 * BSP:(i + 1) * 516],
                     Rt[:, 512:640], "s")

            # software pipeline: PE runs pass1(i+1) before pass2(i)
            Xg = {0: X0, 1: load_x(1)}
            Xv = {g: Xg[g][:].rearrange("p (i q c) -> p i q c", i=GRP, q=4)
                  for g in (0, 1)}
            Yts = {0: None}
            Yts[0] = pass1(Xv[0], 0)
            STG = None
            for i in range(IMGS):
                g, ig = divmod(i, GRP)
                if ig == 0:
                    if g + 2 < NG:
                        Xg[g + 2] = load_x(g + 2)
                        Xv[g + 2] = Xg[g + 2][:].rearrange(
                            "p (i q c) -> p i q c", i=GRP, q=4)
                    STG = spool.tile([128, GRP * 4 * 516], F16, tag="STG")
                if i + 1 < IMGS:
                    g1, ig1 = divmod(i + 1, GRP)
                    Yts[i + 1] = pass1(Xv[g1], ig1)
                pass2(Yts[i], STG, ig, i)
                del Yts[i]
                if ig == GRP - 1:
                    nc.gpsimd.dma_start(out=y_main[g], in_=STG[:])
                if i % 8 == 7:
                    c = i // 8
                    nc.gpsimd.dma_start(
                        out=y_rem[:, c * 8 * 516:(c + 1) * 8 * 516],
                        in_=Crem[:, c * 8 * 516:(c + 1) * 8 * 516])
    nc.finalize()
    return nc


def _get_module() -> bass.Bass:
    global _MODULE
    if _MODULE is None:
        _MODULE = _build_module()
    return _MODULE


def make_in_maps(x: np.ndarray) -> list[dict]:
    x = np.asarray(x, dtype=np.float32)
    B, C, Hx, Wx = x.shape
    assert (Hx, Wx) == (H, W) and B * C == N_CORES * IMGS
    imgs = x.reshape(B * C, H, W)
    maps = []
    for k in range(N_CORES):
        # X[g][p, ig*2048 + q*512 + c] = x[g*GRP+ig, 128q + p, c]
        xc = imgs[k * IMGS:(k + 1) * IMGS].reshape(NG, GRP, 4, 128, W)
        xc = np.ascontiguousarray(xc.transpose(0, 3, 1, 2, 4))
        maps.append({"x": xc.reshape(NG, 128, GRP * 4 * W).astype(np.float16),
                     "w": _WEIGHTS})
    return maps


def kernel(**inputs) -> np.ndarray:
    x = np.asarray(inputs["x"], dtype=np.float32)
    B, C, Hx, Wx = x.shape

    nc = _get_module()
    in_maps = make_in_maps(x)
    res = run_bass_kernel_spmd(nc, in_maps, list(range(N_CORES))).results

    full = np.empty((N_CORES * IMGS, 4, OUT, OUT), dtype=np.float32)
    for k in range(N_CORES):
        # [g, p, ig, khc, f, kw, gg]
        ym = res[k]["y"].reshape(NG, 128, GRP, 2, 2, OUT, 2)
        yr = res[k]["yr"].reshape(4, IMGS, OUT, 2)  # [j*2+f, i, kw, gg]
        dst = full[k * IMGS:(k + 1) * IMGS]
        # dst[g*GRP+ig, f+2gg, khc*128+p, kw] = ym[g, p, ig, khc, f, kw, gg]
        t = ym.transpose(0, 2, 4, 6, 3, 1, 5).reshape(IMGS, 4, 256, OUT)
        # t's dim-1 is f*2+gg; reorder to s = f+2gg -> fg indices [0,2,1,3]
        dst[:, :, :256, :] = t[:, [0, 2, 1, 3]]
        for f in range(2):
            for j in range(2):
                for g in range(2):
                    dst[:, f + 2 * g, 256 + j, :] = yr[j * 2 + f, :, :, g]

    return np.ascontiguousarray(full.reshape(B, 4 * C, OUT, OUT))


# revision 14
# speedup vs baseline: 7.2791x; 1.0125x over previous
"""Trainium2 Bass kernel for 2D single-level DWT (coif1, symmetric padding).

Input  x: (4, 64, 512, 512) fp32
Output  : (4, 256, 258, 258) fp32  -- per input channel: [cA, cH, cV, cD]

Math: with R_f the banded 258x512 operator of the 1D DWT along an axis
(6-tap filter, stride 2, symmetric boundary folds), the four outputs are
    cA = R_lo X R_lo^T,  cH = R_hi X R_lo^T,
    cV = R_lo X R_hi^T,  cD = R_hi X R_hi^T.

v4 design (fp16 data path, band-windowed matmuls, 32 images per core):
  pass 1 (contract rows r):   Yt_f[c, kh] = sum_r X[r, c] R_f[kh, r]
     stationary lhsT = X chunk [r:128, c:128]; moving rhs = R^T slice with
     the lo/hi filter pair interleaved along the stream dim (col 2*kh+f),
     so one matmul serves both filters per LDWEIGHTS.  R is banded:
     r-chunk q only reaches kh in [64q, 64q+66), so each matmul streams
     ~132 interleaved columns instead of 516.
  pass 2 (contract cols c):   O_s[kh, kw] = sum_c Yt_f[c, kh] R_g[kw, c]
     stationary lhsT = Yt chunk (stride-2 slice of the interleaved Yt);
     kh tiled [0,128), [128,256), plus a 2-row remainder whose lhsT is the
     4 contiguous tail columns of each Yt block.
  PSUM accumulation relies on per-element has_written bits: first matmul
  into a bank uses start=True (arms lazy-zero for the whole bank); later
  chain matmuls use start=False and may touch a mix of written
  (accumulate) and pending-zero (overwrite) columns.
  Chains are PAIRED into double-width PSUM tiles (A-pair spans 2 banks,
  B-pair shares 1 bank) so one engine copy drains two chains -- the
  scalar engine pays ~200ns fixed cost per instruction, so fewer, bigger
  drains matter.  The PE runs pass1(i+1) before pass2(i) so drains always
  trail a full chain-group behind the producer (no PSUM-recycle stalls).
  DMA: 2-image granularity, 128 partitions x >=2KiB contiguous per
  partition per transfer (spreads over all 16 SDMA engines).
"""

import os
import sys

for _p in ("/opt/trn_rl_repo", "/opt/pypackages"):
    if _p not in sys.path:
        sys.path.append(_p)

os.environ.setdefault("JAX_COMPILATION_CACHE_DIR", "/tmp/jax_comp_cache")
os.environ.setdefault("JAX_PERSISTENT_CACHE_MIN_COMPILE_TIME_SECS", "10")

import numpy as np

import concourse.bass as bass
import concourse.bacc as bacc
import concourse.mybir as mybir
from concourse.bass_utils import run_bass_kernel_spmd
from concourse.tile import TileContext

N_CORES = 8
H = W = 512
OUT = 258  # (512 + 6 - 1) // 2
IMGS = 32  # images per core (4*64/8)
GRP = 2    # images per DMA transfer
NG = IMGS // GRP
F16 = mybir.dt.float16
F32 = mybir.dt.float32

# pywt coif1 decomposition filters
DEC_LO = np.array([-0.01565572813546454, -0.0727326195128539, 0.38486484686420286,
                   0.8525720202122554, 0.3378976624578092, -0.0727326195128539])
DEC_HI = np.array([0.0727326195128539, 0.3378976624578092, -0.8525720202122554,
                   0.38486484686420286, 0.0727326195128539, -0.01565572813546454])
FLEN = 6
PAD = 4
LO_F = DEC_LO[::-1]
HI_F = DEC_HI[::-1]

# kh/kw window that r/c-chunk q contributes to (from the band structure)
WINS = [(0, 66), (64, 130), (128, 194), (192, 258)]
BSP = 194  # per-chain PSUM split: [0,194)x2 = 1552B (A), [194,258)x2 = 512B (B)

# If True, split matmuls so no instruction touches a mix of
# already-written and pending-zero PSUM bytes (needed only for CoreSim;
# hardware has per-element has_written bits).
INTERP_SAFE = False


def _build_R(filt: np.ndarray, n: int = W) -> np.ndarray:
    """Banded [258, 512] operator: out[k] = sum_j filt[j] * x[sym(2k + j - PAD)]."""
    out_len = (n + FLEN - 1) // 2

    def sym(i: int) -> int:
        while i < 0 or i >= n:
            if i < 0:
                i = -i - 1
            if i >= n:
                i = 2 * n - 1 - i
        return i

    R = np.zeros((out_len, n), dtype=np.float64)
    for k in range(out_len):
        for j in range(FLEN):
            R[k, sym(2 * k + j - PAD)] += filt[j]
    return R


def _check_windows(R: np.ndarray) -> None:
    for q in range(4):
        nz = np.nonzero((R[:, 128 * q:128 * (q + 1)] != 0).any(axis=1))[0]
        assert (int(nz.min()), int(nz.max()) + 1) == WINS[q], (q, nz.min(), nz.max())


def _build_weights() -> np.ndarray:
    """Interleaved: w[p, q*516 + 2k + f] = R_f[k, 128q + p], [128, 4*516] fp16."""
    Rs = [_build_R(LO_F), _build_R(HI_F)]
    _check_windows(Rs[0])
    _check_windows(Rs[1])
    w = np.zeros((128, 4 * 2 * OUT), dtype=np.float32)
    for q in range(4):
        blk = np.zeros((128, OUT, 2), dtype=np.float32)
        for f in range(2):
            blk[:, :, f] = Rs[f][:, 128 * q:128 * (q + 1)].T
        w[:, q * 2 * OUT:(q + 1) * 2 * OUT] = blk.reshape(128, 2 * OUT)
    return w.astype(np.float16)


_WEIGHTS = _build_weights()
_MODULE = None


def _build_module() -> bass.Bass:
    nc = bacc.Bacc("TRN2", target_bir_lowering=False, debug=False)
    x_in = nc.declare_dram_parameter("x", [NG, 128, GRP * 4 * W], F16,
                                     isOutput=False)
    w_in = nc.declare_dram_parameter("w", [128, 4 * 2 * OUT], F16, isOutput=False)
    # y[g, p, ((ig*2 + khc)*2 + f)*516 + 2*kw + gg] = O_{f+2gg}[128*khc + p, kw]
    y_main = nc.declare_dram_parameter("y", [NG, 128, GRP * 4 * 516], F16,
                                       isOutput=True)
    # yr[j*2 + f, i*516 + 2*kw + gg] = O_{f+2gg}[256 + j, kw]
    y_rem = nc.declare_dram_parameter("yr", [4, IMGS * 516], F16, isOutput=True)

    with TileContext(nc) as tc:
        with (
            tc.tile_pool(name="wpool", bufs=1) as wpool,
            tc.tile_pool(name="xpool", bufs=3) as xpool,
            tc.tile_pool(name="ypool", bufs=2) as ypool,
            tc.tile_pool(name="spool", bufs=2) as spool,
            tc.tile_pool(name="rpool", bufs=1) as rpool,
            tc.tile_pool(name="psum", bufs=2, space="PSUM") as pspool,
        ):
            Wt = wpool.tile([128, 4 * 2 * OUT], F16)
            Crem = rpool.tile([4, IMGS * 516], F16)

            def load_x(g):
                X = xpool.tile([128, GRP * 4 * W], F16, tag="X", name=f"X_{g}")
                nc.sync.dma_start(out=X[:], in_=x_in[g])
                return X

            X0 = load_x(0)
            nc.gpsimd.dma_start(out=Wt[:], in_=w_in[:])
            Wr = Wt[:]

            # Tiny PE op consuming the weight DMA so later matmuls depend
            # on it via PE program order.
            warm = pspool.tile([1, 256], F32, tag="pBB", bufs=2)
            nc.tensor.matmul(warm[:, 0:1], lhsT=Wr[:, 0:1], rhs=Wr[:, 0:1],
                             start=True, stop=True)

            def copy(dst, src, eng):
                if eng == "s":
                    nc.scalar.copy(out=dst, in_=src)
                else:
                    nc.vector.tensor_copy(out=dst, in_=src)

            def chain(lhsT_fn, A, B, ha, hb):
                """One banded, filter-interleaved accumulation chain into
                half `ha` of A-pair tile A (512-elem halves = bank-aligned)
                and half `hb` of B-pair tile B (128-elem halves)."""
                if INTERP_SAFE:
                    segs = [(0, 0, 66, 0, True, False),
                            (1, 64, 66, 0, False, False),
                            (1, 66, 130, 0, False, False),
                            (2, 128, 130, 0, False, False),
                            (2, 130, 194, 0, False, False),
                            (3, 192, 194, 0, False, True),
                            (3, 194, 258, 1, True, True)]
                else:
                    segs = [(0, 0, 66, 0, True, False),
                            (1, 64, 130, 0, False, False),
                            (2, 128, 194, 0, False, False),
                            (3, 192, 194, 0, False, True),
                            (3, 194, 258, 1, True, True)]
                for q, lo, hi, t, st, sp in segs:
                    if t == 0:
                        out = A[:, ha * 512 + 2 * lo:ha * 512 + 2 * hi]
                    else:
                        out = B[:, hb * 128 + 2 * (lo - BSP):
                                hb * 128 + 2 * (hi - BSP)]
                    rhs = Wr[:, q * 2 * OUT + 2 * lo:q * 2 * OUT + 2 * hi]
                    nc.tensor.matmul(out, lhsT=lhsT_fn(q), rhs=rhs,
                                     start=st, stop=sp)

            def pair_views(A, B):
                Ah = A[:].rearrange("p (h k) -> p h k", h=2)[:, :, 0:2 * BSP]
                Bh = B[:].rearrange("p (h k) -> p h k", h=2)
                return Ah, Bh

            def pass1(Xv, ig):
                """4 paired chains; returns the interleaved Yt tile
                Yt[p, cc*516 + 2*kh + f]."""
                Yt = ypool.tile([128, 4 * 516], F16, tag="Yt")
                Ytv = Yt[:].rearrange("p (cc k) -> p cc k", cc=4)
                for cp in range(2):  # cc pairs (0,1), (2,3)
                    A = pspool.tile([128, 1024], F32, tag="pAA", bufs=3)
                    B = pspool.tile([128, 256], F32, tag="pBB", bufs=2)
                    for h in range(2):
                        cc = cp * 2 + h
                        chain(lambda q: Xv[:, ig, q, cc * 128:(cc + 1) * 128],
                              A[:], B[:], h, h)
                    Ah, Bh = pair_views(A, B)
                    copy(Ytv[:, 2 * cp:2 * cp + 2, 0:2 * BSP], Ah, "s")
                    copy(Ytv[:, 2 * cp:2 * cp + 2, 2 * BSP:516], Bh, "v")
                return Yt

            def pass2(Yt, STG, ig, i):
                Ytr = Yt[:]
                Ytv4 = Ytr.rearrange("p (cc k f) -> p cc k f", cc=4, f=2)
                Sv = STG[:].rearrange("p (blk k) -> p blk k", k=516)
                for khc in range(2):  # pair over f
                    A = pspool.tile([128, 1024], F32, tag="pAA", bufs=3)
                    B = pspool.tile([128, 256], F32, tag="pBB", bufs=2)
                    for f in range(2):
                        chain(lambda q: Ytv4[:, q, 128 * khc:128 * (khc + 1), f],
                              A[:], B[:], f, f)
                    Ah, Bh = pair_views(A, B)
                    base = (ig * 2 + khc) * 2
                    copy(Sv[:, base:base + 2, 0:2 * BSP], Ah,
                         "s" if khc == 0 else "v")
                    copy(Sv[:, base:base + 2, 2 * BSP:516], Bh, "v")
                # remainder rows kh in {256,257}: lhsT = 4 contiguous tail
                # cols of each Yt block; psum rows j*2+f.  Uses one pAA
                # tile: A part in bank 0, B part at the start of bank 1.
                Rt = pspool.tile([4, 1024], F32, tag="pAA", bufs=3)
                chain(lambda q: Ytr[:, q * 516 + 512:(q + 1) * 516],
                      Rt[:], Rt[:], 0, 4)
                copy(Crem[:, i * 516:i * 516 + 2 * BSP], Rt[:, 0:2 * BSP], "v")
                copy(Crem[:, i * 516 + 2 * BSP:(i + 1) * 516],
                     Rt[:, 512:640], "s")

            # software pipeline: PE runs pass1(i+1) before pass2(i)
            Xg = {0: X0, 1: load_x(1)}
            Xv = {g: Xg[g][:].rearrange("p (i q c) -> p i q c", i=GRP, q=4)
                  for g in (0, 1)}
            Yts = {0: None}
            Yts[0] = pass1(Xv[0], 0)
            STG = None
            for i in range(IMGS):
                g, ig = divmod(i, GRP)
                if ig == 0:
                    if g + 2 < NG:
                        Xg[g + 2] = load_x(g + 2)
                        Xv[g + 2] = Xg[g + 2][:].rearrange(
                            "p (i q c) -> p i q c", i=GRP, q=4)
                    STG = spool.tile([128, GRP * 4 * 516], F16, tag="STG")
                if i + 1 < IMGS:
                    g1, ig1 = divmod(i + 1, GRP)
                    Yts[i + 1] = pass1(Xv[g1], ig1)
                pass2(Yts[i], STG, ig, i)
                del Yts[i]
                if ig == GRP - 1:
                    ring = nc.sync if g == NG - 1 else nc.gpsimd
                    ring.dma_start(out=y_main[g], in_=STG[:])
                if i % 8 == 7:
                    c = i // 8
                    nc.gpsimd.dma_start(
                        out=y_rem[:, c * 8 * 516:(c + 1) * 8 * 516],
                        in_=Crem[:, c * 8 * 516:(c + 1) * 8 * 516])
    nc.finalize()
    return nc


def _get_module() -> bass.Bass:
    global _MODULE
    if _MODULE is None:
        _MODULE = _build_module()
    return _MODULE


def make_in_maps(x: np.ndarray) -> list[dict]:
    x = np.asarray(x, dtype=np.float32)
    B, C, Hx, Wx = x.shape
    assert (Hx, Wx) == (H, W) and B * C == N_CORES * IMGS
    imgs = x.reshape(B * C, H, W)
    maps = []
    for k in range(N_CORES):
        # X[g][p, ig*2048 + q*512 + c] = x[g*GRP+ig, 128q + p, c]
        xc = imgs[k * IMGS:(k + 1) * IMGS].reshape(NG, GRP, 4, 128, W)
        xc = np.ascontiguousarray(xc.transpose(0, 3, 1, 2, 4))
        maps.append({"x": xc.reshape(NG, 128, GRP * 4 * W).astype(np.float16),
                     "w": _WEIGHTS})
    return maps


def kernel(**inputs) -> np.ndarray:
    x = np.asarray(inputs["x"], dtype=np.float32)
    B, C, Hx, Wx = x.shape

    nc = _get_module()
    in_maps = make_in_maps(x)
    res = run_bass_kernel_spmd(nc, in_maps, list(range(N_CORES))).results

    full = np.empty((N_CORES * IMGS, 4, OUT, OUT), dtype=np.float32)
    for k in range(N_CORES):
        # [g, p, ig, khc, f, kw, gg]
        ym = res[k]["y"].reshape(NG, 128, GRP, 2, 2, OUT, 2)
        yr = res[k]["yr"].reshape(4, IMGS, OUT, 2)  # [j*2+f, i, kw, gg]
        dst = full[k * IMGS:(k + 1) * IMGS]
        # dst[g*GRP+ig, f+2gg, khc*128+p, kw] = ym[g, p, ig, khc, f, kw, gg]
        t = ym.transpose(0, 2, 4, 6, 3, 1, 5).reshape(IMGS, 4, 256, OUT)
        # t's dim-1 is f*2+gg; reorder to s = f+2gg -> fg indices [0,2,1,3]
        dst[:, :, :256, :] = t[:, [0, 2, 1, 3]]
        for f in range(2):
            for j in range(2):
                for g in range(2):
                    dst[:, f + 2 * g, 256 + j, :] = yr[j * 2 + f, :, :, g]

    return np.ascontiguousarray(full.reshape(B, 4 * C, OUT, OUT))
